# revision 7
# baseline (speedup 1.0000x reference)
"""Trainium2 Bass kernel for nn_AttentionModule (sparse_attention).

Reference computation:
  q = tanh(einsum('hde,be->hbd', Query, x))          H=8 D=256 E=1536
  k = tanh(einsum('hdf,blf->hbld', Key, bank))       B=64 L=256 F=768
  s = einsum('hbld,hbd->hbl', k, q)  masked softmax over l
  out = LeakyReLU_0.4(einsum('hbl,blf->bhf', attn, bank))

Strategy (hybrid shard: 4 batch-groups x 2 head-groups over 8 cores):
 * Each core owns 16 b's (8 sorted pairs) and 4 heads.  This halves the
   replicated Query/Key DMA vs pure batch-parallel (the serial DMA wire,
   ~0.36 MB/us, is the binding resource): per-core input drops from
   12.9 MB to ~11.3 MB, and the 3.15 MB Query stream lands by ~18 us so
   the score/softmax pipeline never waits on it.
 * Mask compaction: the 0/1 mask keeps <=~152 of 256 bank columns per b;
   the host gathers unmasked columns, sorts b's by count, and pads each
   pair-slot to the max of its 8 ranked b's.  Padding columns get a -1e4
   additive score bias (exp -> 0) via an extra matmul.
 * The dominant k-matmul runs as error-compensated fp8 (e4m3): with
   Key*64 ~ K8 + Kr and bank*16 ~ B8 + Br, kraw = K8B8 + K8Br + KrB8.
   All three terms share one power-of-two scale, folded into the tanh
   eviction's `scale`.  Each product pair is a DoubleRow matmul.
 * k psums for both 128-row d-chunks live in one two-bank PSUM tile so a
   single tanh eviction drains 2*lpp columns (halves Act instruction
   count; per-instruction PSUM access overhead is ~185ns).
 * Narrow dims (b-pair 2, heads 4) ride in the moving dimension: q,
   score, and emb matmuls cost ap_size 16/1/4 per instruction.
 * score/softmax/emb/out for bp0..6 run as hooks inside the k4..k7 head
   loops; bp7's score matmuls ride one head behind k7's evictions, so
   after the last eviction only exp/softmax/emb/out-DMA remain.
 * Softmax skips max-subtraction (|score| < 40, exp in bf16 is safe);
   1/z is broadcast to [f, h] via a ones-matrix matmul and applied with
   LeakyReLU via one DVE multiply.  One small out-DMA per bp.
"""

import os
import numpy as np
import ml_dtypes

import concourse.bass as bass  # noqa: F401
import concourse.mybir as mybir
import concourse.tile as tile
from concourse import bacc, bass_utils

F32 = mybir.dt.float32
F16 = mybir.dt.float16
BF16 = mybir.dt.bfloat16
FP8 = mybir.dt.float8e4
AF = mybir.ActivationFunctionType
DR = mybir.MatmulPerfMode.DoubleRow

H, D, E, F = 8, 256, 1536, 768
B, L = 64, 256
NCORES = 8
GB, GH = 4, 2              # batch groups x head groups
HL = H // GH               # 4 local heads
BPC = B // GB              # 16 b's per core
NBP = BPC // 2             # 8 b-pairs per core
EC, FC, DC = E // 128, F // 128, D // 128   # 12, 6, 2
# Per-bp padded unmasked-column counts (host sorts 64 b's by count; bp_j
# takes ranks [8j, 8j+8)).  Defaults match the fixed harness input.
LPS_DEFAULT = (152, 136, 132, 130, 128, 126, 124, 120)
SK, SB = 64.0, 16.0        # fp8 pre-scales for Key / bank (powers of two)


def _build_program(lps=LPS_DEFAULT):
    assert all(lp % 2 == 0 for lp in lps)
    lhs_ = [lp // 2 for lp in lps]     # l-chunks: two per b
    lpps = [2 * lp for lp in lps]      # (b2, l') columns per (h, dc) group
    kt_cols = 2 * FC * D               # per-h Key cols ([K8, Kr] streams)
    bkt_cols = [2 * FC * w for w in lpps]     # per-bp bankT cols
    bkt_off = np.cumsum([0] + bkt_cols).tolist()
    bkn_off = np.cumsum([0] + [lh for lh in lhs_]).tolist()
    sb_off = np.cumsum([0] + [4 * lh for lh in lhs_]).tolist()
    tanh_scale = 1.0 / (SK * SB)

    nc = bacc.Bacc("TRN2", target_bir_lowering=False, debug=False,
                   enable_asserts=False, num_devices=NCORES)
    qt = nc.dram_tensor("qt", [HL, 128, EC * D], F16, kind="ExternalInput").ap()
    xt = nc.dram_tensor("xt", [128, EC * BPC], F16, kind="ExternalInput").ap()
    kt = nc.dram_tensor("kt", [HL, 128, kt_cols], FP8, kind="ExternalInput").ap()
    bkt = nc.dram_tensor("bkt", [128, bkt_off[-1]], FP8, kind="ExternalInput").ap()
    bkn = nc.dram_tensor("bkn", [bkn_off[-1], 4 * F], BF16, kind="ExternalInput").ap()
    sbias = nc.dram_tensor("sbias", [1, sb_off[-1]], F32, kind="ExternalInput").ap()
    out = nc.dram_tensor("out", [128, NBP * 2 * FC * HL], F16,
                         kind="ExternalOutput").ap()

    with tile.TileContext(nc) as tc:
        with tc.tile_pool(name="const", bufs=1) as cpool, \
             tc.tile_pool(name="weights", bufs=1) as wpool, \
             tc.tile_pool(name="bktp", bufs=1) as bpool, \
             tc.tile_pool(name="bknp", bufs=1) as npool, \
             tc.tile_pool(name="ksb", bufs=1) as kpool, \
             tc.tile_pool(name="small", bufs=4) as spool, \
             tc.tile_pool(name="psK", bufs=2, space="PSUM") as psK, \
             tc.tile_pool(name="psQ", bufs=1, space="PSUM") as psQ, \
             tc.tile_pool(name="psS", bufs=3, space="PSUM") as psS:

            # ---------------- SBUF tiles ----------------------------------
            xt_sb = cpool.tile([128, EC * BPC], F16)
            kt_sb = [wpool.tile([128, kt_cols], FP8, name=f"kt{h}", tag=f"kt{h}")
                     for h in range(HL)]
            qt_sb = [wpool.tile([128, EC * D], F16, name=f"qt{h}", tag=f"qt{h}")
                     for h in range(HL)]
            bkt_t = [bpool.tile([128, bkt_cols[bp]], FP8,
                                name=f"bkt{bp}", tag=f"bkt{bp}")
                     for bp in range(NBP)]
            bkn_t = [npool.tile([lhs_[bp], 4 * F], BF16,
                                name=f"bkn{bp}", tag=f"bkn{bp}")
                     for bp in range(NBP)]
            sb_sb = cpool.tile([1, sb_off[-1]], F32)
            # f16 output: final values are O(1) so f16 (0.05% rel) halves
            # the tail-critical out-DMA transfers; host upcasts
            o2all = cpool.tile([128, NBP * 2 * FC * HL], F16)
            onesb = cpool.tile([1, BPC], F32)
            ones_mat = cpool.tile([lhs_[0], 128], BF16)
            q_sb = cpool.tile([128, 128], F16)

            # ---------------- DMA: priority order -------------------------
            def dma_bkt(bp, s=None):
                o = bkt_off[bp]
                w = bkt_cols[bp]
                if s is None:
                    nc.sync.dma_start(bkt_t[bp][:], bkt[:, o:o + w])
                else:
                    h2 = w // 2
                    nc.sync.dma_start(bkt_t[bp][:, s * h2:(s + 1) * h2],
                                      bkt[:, o + s * h2:o + (s + 1) * h2])

            # kt0 K8-half and bkt0/1 B8-halves first so the warm-phase T1
            # matmuls (k0+k1 interleaved per head) start early
            hk = kt_cols // 2
            nc.sync.dma_start(kt_sb[0][:, 0:hk], kt[0, :, 0:hk])
            dma_bkt(0, 1)
            dma_bkt(1, 1)
            nc.sync.dma_start(kt_sb[0][:, hk:2 * hk], kt[0, :, hk:2 * hk])
            dma_bkt(0, 0)
            dma_bkt(1, 0)
            nc.sync.dma_start(kt_sb[1][:], kt[1])
            nc.sync.dma_start(xt_sb[:], xt)
            nc.vector.memset(onesb[:], 1.0)
            nc.vector.memset(ones_mat[:], 1.0)
            nc.sync.dma_start(kt_sb[2][:], kt[2])
            dma_bkt(2)
            nc.sync.dma_start(kt_sb[3][:], kt[3])
            dma_bkt(3)
            nc.sync.dma_start(qt_sb[0][:], qt[0])
            dma_bkt(4)
            nc.sync.dma_start(qt_sb[1][:], qt[1])
            dma_bkt(5)
            nc.sync.dma_start(qt_sb[2][:], qt[2])
            nc.sync.dma_start(qt_sb[3][:], qt[3])
            nc.sync.dma_start(sb_sb[:], sbias)
            dma_bkt(6)
            dma_bkt(7)
            for bp in range(NBP):
                nc.sync.dma_start(bkn_t[bp][:],
                                  bkn[bkn_off[bp]:bkn_off[bp + 1]])

            # ---------------- score / softmax / emb helpers ---------------
            k_sb = {}

            def score_mms(bp, ps, h):
                """Score matmuls for one head: 8 ap-1 matmuls (+4 bias
                matmuls at h==0).  Accumulates into ps[0:lh, 0:4*HL]."""
                lh, lp, lpp = lhs_[bp], lps[bp], lpps[bp]
                for b2 in range(2):
                    for lc in range(2):
                        col = (b2 * 2 + lc) * HL
                        if h == 0:
                            boff = sb_off[bp] + (b2 * 2 + lc) * lh
                            nc.tensor.matmul(ps[0:lh, col:col + HL],
                                             sb_sb[:, boff:boff + lh],
                                             onesb[:, 0:HL],
                                             start=True, stop=False)
                        for dc in range(DC):
                            nc.tensor.matmul(
                                ps[0:lh, col + h:col + h + 1],
                                k_sb[(bp, h)][:, dc * lpp + b2 * lp +
                                              lc * lh:dc * lpp + b2 * lp +
                                              lc * lh + lh],
                                q_sb[:, (h * DC + dc) * BPC + bp * 2 + b2:
                                     (h * DC + dc) * BPC + bp * 2 + b2 + 1],
                                start=False,
                                stop=(h == HL - 1 and dc == DC - 1))

            exps = {}

            def score_full(bp, ps_tile=None):
                ps = (ps_tile if ps_tile is not None
                      else psS.tile([128, 512], F32, name="mix", tag="mix"))
                for h in range(HL):
                    score_mms(bp, ps, h)
                lh = lhs_[bp]
                exp_t = spool.tile([lhs_[0], 4 * HL], BF16,
                                   name="exp", tag="exp")
                nc.scalar.activation(exp_t[0:lh, :], ps[0:lh, 0:4 * HL],
                                     AF.Exp)
                exps[bp] = (ps, exp_t)

            def rest_part(bp):
                ps, exp_t = exps.pop(bp)
                lh = lhs_[bp]
                # bp4..7 share the psq tile: give each its own z/emb column
                # regions so the four rest chains don't serialize through
                # write-after-read hazards on the same PSUM columns
                zo = 32 + (bp - 4) * 16 if bp >= 4 else 32
                eo = 192 + (bp - 4) * 48 if bp >= 4 else 192
                # z[b2, h]: the ones-MATRIX lhsT emits column sums broadcast
                # across all 128 partitions, so no separate broadcast step
                ev = exp_t[0:lh, :].rearrange("p (b2 lc h) -> p b2 lc h",
                                              b2=2, lc=2)
                for lc in range(2):
                    nc.tensor.matmul(ps[:, zo:zo + 2 * HL],
                                     ones_mat[0:lh, :], ev[:, :, lc],
                                     start=(lc == 0), stop=(lc == 1))
                rz = spool.tile([128, 2 * HL], F32, name="rz", tag="rz")
                nc.vector.reciprocal(rz[:], ps[:, zo:zo + 2 * HL])
                # emb[f, (b2, fc, h)]
                for b2 in range(2):
                    for fc in range(FC):
                        col = eo + (b2 * FC + fc) * HL
                        for lc in range(2):
                            nc.tensor.matmul(
                                ps[:, col:col + HL],
                                bkn_t[bp][:, (b2 * 2 + lc) * F + fc * 128:
                                          (b2 * 2 + lc) * F + fc * 128 + 128],
                                exp_t[0:lh, (b2 * 2 + lc) * HL:
                                      (b2 * 2 + lc + 1) * HL],
                                start=(lc == 0), stop=(lc == 1))
                o1 = spool.tile([128, 2 * FC * HL], F32, name="o1", tag="o1")
                w = 2 * FC * HL
                o2 = o2all[:, bp * w:(bp + 1) * w]
                # LeakyReLU commutes with the positive 1/z: Prelu the raw
                # emb on Act in parallel with z/recip; one DVE mul finishes
                nc.scalar.activation(o1[:], ps[:, eo:eo + w], AF.Prelu,
                                     alpha=0.4)
                vb = rz[:].rearrange(
                    "p (b2 one h) -> p b2 one h", b2=2,
                    one=1).broadcast_to([128, 2, FC, HL])
                nc.vector.tensor_mul(
                    o2.rearrange("p (b2 fc h) -> p b2 fc h", b2=2, fc=FC),
                    o1[:].rearrange("p (b2 fc h) -> p b2 fc h", b2=2, fc=FC),
                    vb)
                # per-bp out-DMA: earlier bps stream out mid-kernel, only
                # bp7's small transfer sits on the tail
                nc.sync.dma_start(out[:, bp * w:(bp + 1) * w], o2)

            # ---------------- k = tanh(Key @ bankT) -----------------------
            def k_phase(bps, warm=False, hooks=None, tail_bp=None):
                def t1_mms(bp, h, ps2):
                    lpp = lpps[bp]
                    vb = bkt_t[bp][:].rearrange("p (s ft c) -> p s ft c",
                                                s=2, ft=FC)
                    vk = kt_sb[h][:].rearrange("p (s ft d) -> p s ft d",
                                               s=2, ft=FC)
                    for dc in range(DC):
                        g = ps2[:, dc * 512:dc * 512 + lpp]
                        for p in range(FC // 2):
                            nc.tensor.matmul(
                                g,
                                vk[:, 0, 2 * p:2 * p + 2,
                                   dc * 128:(dc + 1) * 128],
                                vb[:, 1, 2 * p:2 * p + 2],
                                start=(p == 0), stop=False, perf_mode=DR)

                def cross_evict(bp, h, ps2):
                    lpp = lpps[bp]
                    vb = bkt_t[bp][:].rearrange("p (s ft c) -> p s ft c",
                                                s=2, ft=FC)
                    vk = kt_sb[h][:].rearrange("p (s ft d) -> p s ft d",
                                               s=2, ft=FC)
                    kt_out = kpool.tile([128, 2 * lpp], F16,
                                        name=f"k{bp}_{h}", tag=f"k{bp}_{h}")
                    for dc in range(DC):
                        g = ps2[:, dc * 512:dc * 512 + lpp]
                        # cross terms: K8.Br + Kr.B8 per f-tile
                        for ft in range(FC):
                            nc.tensor.matmul(
                                g, vk[:, :, ft, dc * 128:(dc + 1) * 128],
                                vb[:, :, ft],
                                start=False, stop=(ft == FC - 1),
                                perf_mode=DR)
                    # one eviction drains both d-chunks: the [128, 2, lpp]
                    # AP walks the two PSUM banks of the ps2 tile
                    nc.scalar.activation(
                        kt_out[:].rearrange("p (dc c) -> p dc c", dc=2),
                        ps2[:].rearrange("p (dc c) -> p dc c",
                                         dc=2)[:, :, 0:lpp],
                        AF.Tanh, scale=tanh_scale)
                    k_sb[(bp, h)] = kt_out

                def new_ps():
                    return psK.tile([128, 1024], F32, name="psk", tag="psk")

                start_h = 0
                if warm:
                    # bp0's first two heads: T1s lead (they need only the
                    # K8/B8 slices), crosses follow, then later bps join
                    tiles = {}
                    for h in range(2):
                        tiles[h] = new_ps()
                        t1_mms(bps[0], h, tiles[h])
                    for h in range(2):
                        cross_evict(bps[0], h, tiles.pop(h))
                    for bp in bps[1:]:
                        for h in range(2):
                            ps2 = new_ps()
                            t1_mms(bp, h, ps2)
                            cross_evict(bp, h, ps2)
                    start_h = 2
                for h in range(start_h, HL):
                    for bp in bps:
                        ps2 = new_ps()
                        t1_mms(bp, h, ps2)
                        cross_evict(bp, h, ps2)
                        if tail_bp is not None and h >= 1:
                            # bp7's score rides one head behind its own
                            # evictions (the h-1 eviction finished while
                            # this head's 18 matmuls ran)
                            score_mms(tail_bp, psq, h - 1)
                    if hooks and h in hooks:
                        for fn in hooks[h]:
                            fn()

            # warm phase: k0+k1 interleaved per head (2.3us of PE work per
            # kt[h] arrival so the lead-in is never DMA-starved)
            k_phase([0, 1], warm=True)
            k_phase([2])
            k_phase([3])

            # ---------------- q = tanh(Query @ x) -------------------------
            # qt (3.15 MB) has streamed in behind the k inputs by now
            psq = psQ.tile([128, 512], F32)
            for h in range(HL):
                vq = qt_sb[h][:].rearrange("p (ec d) -> p ec d", ec=EC)
                for dc in range(DC):
                    g = psq[:, (h * DC + dc) * BPC:(h * DC + dc + 1) * BPC]
                    for ec in range(EC):
                        nc.tensor.matmul(
                            g, vq[:, ec, dc * 128:(dc + 1) * 128],
                            xt_sb[:, ec * BPC:(ec + 1) * BPC],
                            start=(ec == 0), stop=(ec == EC - 1))
            nc.scalar.activation(q_sb[:], psq[:, 0:128], AF.Tanh)

            # ---------------- k4..k7 with score/softmax hooks -------------
            # bp4..7's own score matmuls ride one head behind their own
            # phase's evictions (tail_bp); fin(bp) = last head + exp at the
            # next phase's start.  bp0..3 score/softmax spread over k4's
            # slots, rests two slots after their exp.
            def fin(bp):
                score_mms(bp, psq, HL - 1)
                lh = lhs_[bp]
                exp_t = spool.tile([lhs_[0], 4 * HL], BF16,
                                   name="exp", tag="exp")
                nc.scalar.activation(exp_t[0:lh, :], psq[0:lh, 0:4 * HL],
                                     AF.Exp)
                exps[bp] = (psq, exp_t)

            k_phase([4], tail_bp=4,
                    hooks={0: [lambda: score_full(0)],
                           1: [lambda: score_full(1)],
                           2: [lambda: rest_part(0), lambda: score_full(2)],
                           3: [lambda: rest_part(1), lambda: score_full(3)]})
            k_phase([5], tail_bp=5,
                    hooks={0: [lambda: fin(4), lambda: rest_part(2)],
                           1: [lambda: rest_part(3)],
                           2: [lambda: rest_part(4)]})
            k_phase([6], tail_bp=6,
                    hooks={0: [lambda: fin(5)],
                           1: [lambda: rest_part(5)]})
            k_phase([7], tail_bp=7,
                    hooks={0: [lambda: fin(6)],
                           1: [lambda: rest_part(6)]})
            # tail: bp7's last head, exp, softmax/emb/out
            fin(7)
            rest_part(7)

    nc.finalize()
    return nc


def _slot_plan(mask):
    """Sort b's by unmasked count (desc); bp_j takes ranks [8j, 8j+8).
    Returns (perm, lps): perm[slot] = original b, slot = gb*BPC + j*2 + b2."""
    counts = mask.sum(axis=1)
    order = np.argsort(-counts, kind="stable")
    perm = np.empty(B, dtype=np.int64)
    for j in range(NBP):
        grp = order[8 * j:8 * (j + 1)]
        for gb in range(GB):
            perm[gb * BPC + j * 2] = grp[2 * gb]
            perm[gb * BPC + j * 2 + 1] = grp[2 * gb + 1]
    lps = tuple(max(int(2 * ((counts[order[8 * j]] + 1) // 2)), 8)
                for j in range(NBP))
    return perm, lps


def _host_prep(x, bank, mask, Query, Key, perm, lps):
    x = np.asarray(x, dtype=np.float32)
    bank = np.asarray(bank, dtype=np.float32)
    mask = np.asarray(mask)
    Query = np.asarray(Query, dtype=np.float32)
    Key = np.asarray(Key, dtype=np.float32)
    e4 = ml_dtypes.float8_e4m3
    lhs_ = [lp // 2 for lp in lps]

    # q path: f16, host-transposed; per head-group slice
    xs = x[perm]
    qt_full = np.ascontiguousarray(Query.transpose(0, 2, 1)).reshape(
        H, EC, 128, D).transpose(0, 2, 1, 3).reshape(H, 128, EC * D)
    qt_full = qt_full.astype(np.float16)

    Ks = Key * SK
    K8 = Ks.astype(e4)
    Kr = (Ks - K8.astype(np.float32)).astype(e4)

    def swz_key(Kt):  # [H, D, F] -> [H, 128(f), FC, D]
        t = np.ascontiguousarray(Kt.transpose(0, 2, 1))
        return t.reshape(H, FC, 128, D).transpose(0, 2, 1, 3)

    kt_full = np.stack([swz_key(K8.astype(np.float32)),
                        swz_key(Kr.astype(np.float32))], axis=2)
    kt_full = kt_full.reshape(H, 128, 2 * FC * D).astype(e4)

    # per-(batch-group, bp) compacted bank streams
    bkt_cols = sum(2 * FC * 2 * lp for lp in lps)
    gb_data = []
    for gb in range(GB):
        bkt_c = np.zeros((128, bkt_cols), dtype=e4)
        bkn_rows = []
        sb_c = []
        col = 0
        for j in range(NBP):
            lp, lh = lps[j], lhs_[j]
            bc = np.zeros((2, lp, F), dtype=np.float32)
            bias = np.zeros((2, lp), dtype=np.float32)
            for b2 in range(2):
                bsrc = perm[gb * BPC + j * 2 + b2]
                idx = np.nonzero(mask[bsrc])[0]
                bc[b2, :len(idx)] = bank[bsrc, idx]
                bias[b2, len(idx):] = -10000.0
            # bankT swizzle: [2, lp, F] -> [128(f), s, FC, 2, lp]
            t = np.ascontiguousarray(bc.transpose(0, 2, 1))     # [2, F, lp]
            t = t.reshape(2, FC, 128, lp).transpose(2, 1, 0, 3)  # [128,FC,2,lp]
            ts = t * SB
            t8 = ts.astype(e4)
            tr = (ts - t8.astype(np.float32)).astype(e4)
            blk = np.stack([tr, t8.astype(e4)], axis=1).reshape(
                128, 2 * FC * 2 * lp)
            w = 2 * FC * 2 * lp
            bkt_c[:, col:col + w] = blk
            col += w
            # bkn rows [lh, (b2, lc, F)]
            bkn_rows.append(bc.reshape(2, 2, lh, F).transpose(2, 0, 1, 3)
                            .reshape(lh, 4 * F))
            sb_c.append(bias.reshape(4 * lh))
        xt_gb = np.ascontiguousarray(
            xs[gb * BPC:(gb + 1) * BPC].T.reshape(EC, 128, BPC)
            .transpose(1, 0, 2).reshape(128, EC * BPC)).astype(np.float16)
        gb_data.append({
            "xt": xt_gb,
            "bkt": bkt_c,
            "bkn": np.ascontiguousarray(np.concatenate(bkn_rows, axis=0))
            .astype(ml_dtypes.bfloat16),
            "sbias": np.concatenate(sb_c)[None, :].astype(np.float32),
        })

    in_maps = []
    for c in range(NCORES):
        gb, gh = c // GH, c % GH
        m = dict(gb_data[gb])
        m["qt"] = qt_full[gh * HL:(gh + 1) * HL]
        m["kt"] = kt_full[gh * HL:(gh + 1) * HL]
        in_maps.append(m)
    return in_maps


_NC_CACHE = {}


def kernel(x, bank, mask, Query, Key):
    mask = np.asarray(mask)
    perm, lps = _slot_plan(mask)
    if lps not in _NC_CACHE:
        _NC_CACHE[lps] = _build_program(lps)
    nc = _NC_CACHE[lps]
    in_maps = _host_prep(x, bank, mask, Query, Key, perm, lps)

    trace = os.environ.get("KERNEL_TRACE", "0") == "1"
    res = bass_utils.run_bass_kernel_spmd(nc, in_maps,
                                          core_ids=list(range(NCORES)),
                                          trace=trace)
    if trace:
        print("exec_time_ns:", res.exec_time_ns,
              "mean:", res.mean_exec_time_ns,
              "core:", res.max_exec_time_core_id)
    full = np.empty((B, H, F), dtype=np.float32)
    for c, r in enumerate(res.results):
        gb, gh = c // GH, c % GH
        a = r["out"].astype(np.float32).reshape(128, NBP, 2, FC, HL)
        # [p, j, b2, fc, h] -> [(j b2), h, (fc p)]
        full[perm[gb * BPC:(gb + 1) * BPC], gh * HL:(gh + 1) * HL] = (
            a.transpose(1, 2, 4, 3, 0).reshape(BPC, HL, F))
    return np.ascontiguousarray(full)


# revision 19
# speedup vs baseline: 1.0381x; 1.0381x over previous
"""Trainium2 Bass kernel for nn_AttentionModule (sparse_attention).

Reference computation:
  q = tanh(einsum('hde,be->hbd', Query, x))          H=8 D=256 E=1536
  k = tanh(einsum('hdf,blf->hbld', Key, bank))       B=64 L=256 F=768
  s = einsum('hbld,hbd->hbl', k, q)  masked softmax over l
  out = LeakyReLU_0.4(einsum('hbl,blf->bhf', attn, bank))

Strategy (hybrid shard: 4 batch-groups x 2 head-groups over 8 cores):
 * Each core owns 16 b's (8 sorted pairs) and 4 heads.  This halves the
   replicated Query/Key DMA vs pure batch-parallel (the serial DMA wire,
   ~0.36 MB/us, is the binding resource): per-core input drops from
   12.9 MB to ~11.3 MB, and the 3.15 MB Query stream lands by ~18 us so
   the score/softmax pipeline never waits on it.
 * Mask compaction: the 0/1 mask keeps <=~152 of 256 bank columns per b;
   the host gathers unmasked columns, sorts b's by count, and pads each
   pair-slot to the max of its 8 ranked b's.  Padding columns get a -1e4
   additive score bias (exp -> 0) via an extra matmul.
 * The dominant k-matmul runs as error-compensated fp8 (e4m3): with
   Key*64 ~ K8 + Kr and bank*16 ~ B8 + Br, kraw = K8B8 + K8Br + KrB8.
   All three terms share one power-of-two scale, folded into the tanh
   eviction's `scale`.  Each product pair is a DoubleRow matmul.
 * k psums for both 128-row d-chunks live in one two-bank PSUM tile so a
   single tanh eviction drains 2*lpp columns (halves Act instruction
   count; per-instruction PSUM access overhead is ~185ns).
 * Narrow dims (b-pair 2, heads 4) ride in the moving dimension: q,
   score, and emb matmuls cost ap_size 16/1/4 per instruction.
 * score/softmax/emb/out for bp0..6 run as hooks inside the k4..k7 head
   loops; bp7's score matmuls ride one head behind k7's evictions, so
   after the last eviction only exp/softmax/emb/out-DMA remain.
 * Softmax skips max-subtraction (|score| < 40, exp in bf16 is safe);
   1/z is broadcast to [f, h] via a ones-matrix matmul and applied with
   LeakyReLU via one DVE multiply.  One small out-DMA per bp.
"""

import os
import numpy as np
import ml_dtypes

import concourse.bass as bass  # noqa: F401
import concourse.mybir as mybir
import concourse.tile as tile
from concourse import bacc, bass_utils

F32 = mybir.dt.float32
F16 = mybir.dt.float16
BF16 = mybir.dt.bfloat16
FP8 = mybir.dt.float8e4
AF = mybir.ActivationFunctionType
DR = mybir.MatmulPerfMode.DoubleRow

H, D, E, F = 8, 256, 1536, 768
B, L = 64, 256
NCORES = 8
GB, GH = 4, 2              # batch groups x head groups
HL = H // GH               # 4 local heads
BPC = B // GB              # 16 b's per core
NBP = BPC // 2             # 8 b-pairs per core
EC, FC, DC = E // 128, F // 128, D // 128   # 12, 6, 2
# Per-bp padded unmasked-column counts (host sorts 64 b's by count; bp_j
# takes ranks [8j, 8j+8)).  Defaults match the fixed harness input.
LPS_DEFAULT = (152, 136, 132, 130, 128, 126, 124, 120)
SK, SB = 64.0, 16.0        # fp8 pre-scales for Key / bank (powers of two)


def _build_program(lps=LPS_DEFAULT):
    assert all(lp % 2 == 0 for lp in lps)
    lhs_ = [lp // 2 for lp in lps]     # l-chunks: two per b
    lpps = [2 * lp for lp in lps]      # (b2, l') columns per (h, dc) group
    kt_cols = 2 * FC * D               # per-h Key cols ([K8, Kr] streams)
    bkt_cols = [2 * FC * w for w in lpps]     # per-bp bankT cols
    bkt_off = np.cumsum([0] + bkt_cols).tolist()
    bkn_off = np.cumsum([0] + [lh for lh in lhs_]).tolist()
    sb_off = np.cumsum([0] + [4 * lh for lh in lhs_]).tolist()
    tanh_scale = 1.0 / (SK * SB)

    nc = bacc.Bacc("TRN2", target_bir_lowering=False, debug=False,
                   enable_asserts=False, num_devices=NCORES)
    qt = nc.dram_tensor("qt", [HL, 128, EC * D], F16, kind="ExternalInput").ap()
    xt = nc.dram_tensor("xt", [128, EC * BPC], F16, kind="ExternalInput").ap()
    kt = nc.dram_tensor("kt", [HL, 128, kt_cols], FP8, kind="ExternalInput").ap()
    bkt = nc.dram_tensor("bkt", [128, bkt_off[-1]], FP8, kind="ExternalInput").ap()
    bkn = nc.dram_tensor("bkn", [bkn_off[-1], 4 * F], BF16, kind="ExternalInput").ap()
    sbias = nc.dram_tensor("sbias", [1, sb_off[-1]], F32, kind="ExternalInput").ap()
    out = nc.dram_tensor("out", [128, NBP * 2 * FC * HL], F16,
                         kind="ExternalOutput").ap()

    with tile.TileContext(nc) as tc:
        with tc.tile_pool(name="const", bufs=1) as cpool, \
             tc.tile_pool(name="weights", bufs=1) as wpool, \
             tc.tile_pool(name="bktp", bufs=1) as bpool, \
             tc.tile_pool(name="bknp", bufs=1) as npool, \
             tc.tile_pool(name="ksb", bufs=1) as kpool, \
             tc.tile_pool(name="small", bufs=4) as spool, \
             tc.tile_pool(name="outp", bufs=NBP) as opool, \
             tc.tile_pool(name="psK", bufs=2, space="PSUM") as psK, \
             tc.tile_pool(name="psS", bufs=2, space="PSUM") as psS, \
             tc.tile_pool(name="psT", bufs=2, space="PSUM") as psT:

            # ---------------- SBUF tiles ----------------------------------
            xt_sb = cpool.tile([128, EC * BPC], F16)
            kt_sb = [wpool.tile([128, kt_cols], FP8, name=f"kt{h}", tag=f"kt{h}")
                     for h in range(HL)]
            qt_sb = [wpool.tile([128, EC * D], F16, name=f"qt{h}", tag=f"qt{h}")
                     for h in range(HL)]
            bkt_t = [bpool.tile([128, bkt_cols[bp]], FP8,
                                name=f"bkt{bp}", tag=f"bkt{bp}")
                     for bp in range(NBP)]
            bkn_t = [npool.tile([lhs_[bp], 4 * F], BF16,
                                name=f"bkn{bp}", tag=f"bkn{bp}")
                     for bp in range(NBP)]
            sb_sb = cpool.tile([1, sb_off[-1]], F32)
            onesb = cpool.tile([1, BPC], F32)
            ones_mat = cpool.tile([lhs_[0], 128], BF16)
            q_sb = cpool.tile([128, 128], F16)

            # ---------------- DMA: priority order -------------------------
            def dma_bkt(bp, s=None):
                o = bkt_off[bp]
                w = bkt_cols[bp]
                if s is None:
                    nc.sync.dma_start(bkt_t[bp][:], bkt[:, o:o + w])
                else:
                    h2 = w // 2
                    nc.sync.dma_start(bkt_t[bp][:, s * h2:(s + 1) * h2],
                                      bkt[:, o + s * h2:o + (s + 1) * h2])

            # kt0/kt1 K8-halves and bkt0/1 B8-halves first so the warm-phase
            # T1 matmuls (k0's h0/h1 lead, then k1 joins) start early; the
            # residual (Kr/Br) streams follow for the cross terms
            hk = kt_cols // 2
            nc.sync.dma_start(kt_sb[0][:, 0:hk], kt[0, :, 0:hk])
            dma_bkt(0, 1)
            dma_bkt(1, 1)
            nc.sync.dma_start(kt_sb[1][:, 0:hk], kt[1, :, 0:hk])
            nc.sync.dma_start(kt_sb[0][:, hk:2 * hk], kt[0, :, hk:2 * hk])
            dma_bkt(0, 0)
            dma_bkt(1, 0)
            nc.sync.dma_start(kt_sb[1][:, hk:2 * hk], kt[1, :, hk:2 * hk])
            nc.sync.dma_start(xt_sb[:], xt)
            nc.vector.memset(onesb[:], 1.0)
            nc.vector.memset(ones_mat[:], 1.0)
            nc.sync.dma_start(kt_sb[2][:], kt[2])
            dma_bkt(2)
            nc.sync.dma_start(kt_sb[3][:], kt[3])
            dma_bkt(3)
            nc.sync.dma_start(qt_sb[0][:], qt[0])
            dma_bkt(4)
            nc.sync.dma_start(qt_sb[1][:], qt[1])
            dma_bkt(5)
            nc.sync.dma_start(qt_sb[2][:], qt[2])
            nc.sync.dma_start(qt_sb[3][:], qt[3])
            nc.sync.dma_start(sb_sb[:], sbias)
            dma_bkt(6)
            dma_bkt(7)
            for bp in range(NBP):
                nc.sync.dma_start(bkn_t[bp][:],
                                  bkn[bkn_off[bp]:bkn_off[bp + 1]])

            # ---------------- score / softmax / emb helpers ---------------
            k_sb = {}

            # Dependency tracking is TILE-granular: any write of a PSUM tile
            # serializes against all prior reads of that tile.  So every bp
            # gets its OWN [128, 512] score/z/emb tile (bp0..3 rotate psS,
            # bp4..7 rotate psT) and the 8 softmax chains pipeline freely.
            sc_ps = {}

            def score_tile(bp):
                if bp not in sc_ps:
                    pool = psT if bp >= 4 else psS
                    sc_ps[bp] = pool.tile([128, 512], F32,
                                          name="mix", tag="mix")
                return sc_ps[bp]

            def score_mms(bp, h):
                """Score matmuls for one head: 8 ap-1 matmuls (+4 bias
                matmuls at h==0).  Accumulates into ps[0:lh, 0:4*HL]."""
                ps = score_tile(bp)
                lh, lp, lpp = lhs_[bp], lps[bp], lpps[bp]
                for b2 in range(2):
                    for lc in range(2):
                        col = (b2 * 2 + lc) * HL
                        if h == 0:
                            boff = sb_off[bp] + (b2 * 2 + lc) * lh
                            nc.tensor.matmul(ps[0:lh, col:col + HL],
                                             sb_sb[:, boff:boff + lh],
                                             onesb[:, 0:HL],
                                             start=True, stop=False)
                        for dc in range(DC):
                            nc.tensor.matmul(
                                ps[0:lh, col + h:col + h + 1],
                                k_sb[(bp, h)][:, dc * lpp + b2 * lp +
                                              lc * lh:dc * lpp + b2 * lp +
                                              lc * lh + lh],
                                q_sb[:, (h * DC + dc) * BPC + bp * 2 + b2:
                                     (h * DC + dc) * BPC + bp * 2 + b2 + 1],
                                start=False,
                                stop=(h == HL - 1 and dc == DC - 1))

            exps = {}

            def score_exp(bp):
                ps = sc_ps[bp]
                lh = lhs_[bp]
                exp_t = spool.tile([lhs_[0], 4 * HL], BF16,
                                   name="exp", tag="exp")
                nc.scalar.activation(exp_t[0:lh, :], ps[0:lh, 0:4 * HL],
                                     AF.Exp)
                exps[bp] = (ps, exp_t)

            def score_full(bp):
                for h in range(HL):
                    score_mms(bp, h)
                score_exp(bp)

            def rest_part(bp):
                ps, exp_t = exps.pop(bp)
                sc_ps.pop(bp)
                lh = lhs_[bp]
                # z[b2, h] (cols 32:40): the ones-MATRIX lhsT emits column
                # sums broadcast across all 128 partitions, so no separate
                # broadcast step is needed.  Both matmul groups (z, emb) are
                # issued back-to-back on PE; recip and prelu are then both
                # READS of ps and run concurrently (tile-granular deps).
                ev = exp_t[0:lh, :].rearrange("p (b2 lc h) -> p b2 lc h",
                                              b2=2, lc=2)
                for lc in range(2):
                    nc.tensor.matmul(ps[:, 32:32 + 2 * HL],
                                     ones_mat[0:lh, :], ev[:, :, lc],
                                     start=(lc == 0), stop=(lc == 1))
                # emb[f, (b2, fc, h)] (cols 192:240)
                for b2 in range(2):
                    for fc in range(FC):
                        col = 192 + (b2 * FC + fc) * HL
                        for lc in range(2):
                            nc.tensor.matmul(
                                ps[:, col:col + HL],
                                bkn_t[bp][:, (b2 * 2 + lc) * F + fc * 128:
                                          (b2 * 2 + lc) * F + fc * 128 + 128],
                                exp_t[0:lh, (b2 * 2 + lc) * HL:
                                      (b2 * 2 + lc + 1) * HL],
                                start=(lc == 0), stop=(lc == 1))
                rz = spool.tile([128, 2 * HL], F32, name="rz", tag="rz")
                nc.vector.reciprocal(rz[:], ps[:, 32:32 + 2 * HL])
                o1 = spool.tile([128, 2 * FC * HL], F32, name="o1", tag="o1")
                w = 2 * FC * HL
                # LeakyReLU commutes with the positive 1/z: Prelu the raw
                # emb on Act in parallel with the recip; one DVE mul ends
                nc.scalar.activation(o1[:], ps[:, 192:192 + w], AF.Prelu,
                                     alpha=0.4)
                # per-bp SBUF out tile: sharing one o2 tile would serialize
                # the muls behind earlier bps' out-DMA reads
                o2 = opool.tile([128, w], F16, name=f"o2_{bp}",
                                tag=f"o2_{bp}")
                vb = rz[:].rearrange(
                    "p (b2 one h) -> p b2 one h", b2=2,
                    one=1).broadcast_to([128, 2, FC, HL])
                nc.vector.tensor_mul(
                    o2[:].rearrange("p (b2 fc h) -> p b2 fc h", b2=2, fc=FC),
                    o1[:].rearrange("p (b2 fc h) -> p b2 fc h", b2=2, fc=FC),
                    vb)
                # per-bp out-DMA: earlier bps stream out mid-kernel, only
                # bp7's small transfer sits on the tail
                nc.sync.dma_start(out[:, bp * w:(bp + 1) * w], o2[:])

            # ---------------- k = tanh(Key @ bankT) -----------------------
            def k_phase(bps, warm=False, hooks=None, tail_bp=None):
                def t1_mms(bp, h, ps2):
                    lpp = lpps[bp]
                    vb = bkt_t[bp][:].rearrange("p (s ft c) -> p s ft c",
                                                s=2, ft=FC)
                    vk = kt_sb[h][:].rearrange("p (s ft d) -> p s ft d",
                                               s=2, ft=FC)
                    for dc in range(DC):
                        g = ps2[:, dc * 512:dc * 512 + lpp]
                        for p in range(FC // 2):
                            nc.tensor.matmul(
                                g,
                                vk[:, 0, 2 * p:2 * p + 2,
                                   dc * 128:(dc + 1) * 128],
                                vb[:, 1, 2 * p:2 * p + 2],
                                start=(p == 0), stop=False, perf_mode=DR)

                def cross_evict(bp, h, ps2):
                    lpp = lpps[bp]
                    vb = bkt_t[bp][:].rearrange("p (s ft c) -> p s ft c",
                                                s=2, ft=FC)
                    vk = kt_sb[h][:].rearrange("p (s ft d) -> p s ft d",
                                               s=2, ft=FC)
                    kt_out = kpool.tile([128, 2 * lpp], F16,
                                        name=f"k{bp}_{h}", tag=f"k{bp}_{h}")
                    for dc in range(DC):
                        g = ps2[:, dc * 512:dc * 512 + lpp]
                        # cross terms: K8.Br + Kr.B8 per f-tile
                        for ft in range(FC):
                            nc.tensor.matmul(
                                g, vk[:, :, ft, dc * 128:(dc + 1) * 128],
                                vb[:, :, ft],
                                start=False, stop=(ft == FC - 1),
                                perf_mode=DR)
                    # one eviction drains both d-chunks: the [128, 2, lpp]
                    # AP walks the two PSUM banks of the ps2 tile
                    nc.scalar.activation(
                        kt_out[:].rearrange("p (dc c) -> p dc c", dc=2),
                        ps2[:].rearrange("p (dc c) -> p dc c",
                                         dc=2)[:, :, 0:lpp],
                        AF.Tanh, scale=tanh_scale)
                    k_sb[(bp, h)] = kt_out

                def new_ps():
                    return psK.tile([128, 1024], F32, name="psk", tag="psk")

                start_h = 0
                if warm:
                    # bp0's first two heads: T1s lead (they need only the
                    # K8/B8 slices), crosses follow, then later bps join
                    tiles = {}
                    for h in range(2):
                        tiles[h] = new_ps()
                        t1_mms(bps[0], h, tiles[h])
                    for h in range(2):
                        cross_evict(bps[0], h, tiles.pop(h))
                    for bp in bps[1:]:
                        for h in range(2):
                            ps2 = new_ps()
                            t1_mms(bp, h, ps2)
                            cross_evict(bp, h, ps2)
                    start_h = 2
                for h in range(start_h, HL):
                    for bp in bps:
                        ps2 = new_ps()
                        t1_mms(bp, h, ps2)
                        cross_evict(bp, h, ps2)
                        if tail_bp is not None and h >= 1:
                            # the phase's own bp's score rides one head
                            # behind its evictions (the h-1 eviction ended
                            # while this head's 18 matmuls ran)
                            score_mms(tail_bp, h - 1)
                    if hooks and h in hooks:
                        for fn in hooks[h]:
                            fn()

            # warm phase: k0+k1 interleaved per head (2.3us of PE work per
            # kt[h] arrival so the lead-in is never DMA-starved)
            k_phase([0, 1], warm=True)
            k_phase([2])
            k_phase([3])

            # ---------------- q = tanh(Query @ x) -------------------------
            # qt (3.15 MB) has streamed in behind the k inputs by now.
            # The q psum borrows a psS buffer (freed at the tanh eviction,
            # before bp1's score needs it).
            psq = psS.tile([128, 512], F32, name="mix", tag="mix")
            for h in range(HL):
                vq = qt_sb[h][:].rearrange("p (ec d) -> p ec d", ec=EC)
                for dc in range(DC):
                    g = psq[:, (h * DC + dc) * BPC:(h * DC + dc + 1) * BPC]
                    for ec in range(EC):
                        nc.tensor.matmul(
                            g, vq[:, ec, dc * 128:(dc + 1) * 128],
                            xt_sb[:, ec * BPC:(ec + 1) * BPC],
                            start=(ec == 0), stop=(ec == EC - 1))
            nc.scalar.activation(q_sb[:], psq[:, 0:128], AF.Tanh)

            # ---------------- k4..k7 with score/softmax hooks -------------
            # bp4..7's own score matmuls ride one head behind their own
            # phase's evictions (tail_bp); fin(bp) = last head + exp at the
            # next phase's start.  bp0..3 score/softmax spread over k4/k5's
            # slots, rests two slots after their exp.
            def fin(bp):
                score_mms(bp, HL - 1)
                score_exp(bp)

            k_phase([4], tail_bp=4,
                    hooks={0: [lambda: score_full(0)],
                           1: [lambda: score_full(1)],
                           2: [lambda: rest_part(0)],
                           3: [lambda: rest_part(1)]})
            k_phase([5], tail_bp=5,
                    hooks={0: [lambda: fin(4), lambda: score_full(2)],
                           1: [lambda: score_full(3)],
                           2: [lambda: rest_part(2), lambda: rest_part(4)],
                           3: [lambda: rest_part(3)]})
            k_phase([6], tail_bp=6,
                    hooks={0: [lambda: fin(5)],
                           2: [lambda: rest_part(5)]})
            k_phase([7], tail_bp=7,
                    hooks={0: [lambda: fin(6)],
                           2: [lambda: rest_part(6)]})
            # tail: bp7's last head, exp, softmax/emb/out
            fin(7)
            rest_part(7)

    nc.finalize()
    return nc


def _slot_plan(mask):
    """Sort b's by unmasked count (desc); bp_j takes ranks [8j, 8j+8).
    Returns (perm, lps): perm[slot] = original b, slot = gb*BPC + j*2 + b2."""
    counts = mask.sum(axis=1)
    order = np.argsort(-counts, kind="stable")
    perm = np.empty(B, dtype=np.int64)
    for j in range(NBP):
        grp = order[8 * j:8 * (j + 1)]
        for gb in range(GB):
            perm[gb * BPC + j * 2] = grp[2 * gb]
            perm[gb * BPC + j * 2 + 1] = grp[2 * gb + 1]
    lps = tuple(max(int(2 * ((counts[order[8 * j]] + 1) // 2)), 8)
                for j in range(NBP))
    return perm, lps


def _host_prep(x, bank, mask, Query, Key, perm, lps):
    x = np.asarray(x, dtype=np.float32)
    bank = np.asarray(bank, dtype=np.float32)
    mask = np.asarray(mask)
    Query = np.asarray(Query, dtype=np.float32)
    Key = np.asarray(Key, dtype=np.float32)
    e4 = ml_dtypes.float8_e4m3
    lhs_ = [lp // 2 for lp in lps]

    # q path: f16, host-transposed; per head-group slice
    xs = x[perm]
    qt_full = np.ascontiguousarray(Query.transpose(0, 2, 1)).reshape(
        H, EC, 128, D).transpose(0, 2, 1, 3).reshape(H, 128, EC * D)
    qt_full = qt_full.astype(np.float16)

    Ks = Key * SK
    K8 = Ks.astype(e4)
    Kr = (Ks - K8.astype(np.float32)).astype(e4)

    def swz_key(Kt):  # [H, D, F] -> [H, 128(f), FC, D]
        t = np.ascontiguousarray(Kt.transpose(0, 2, 1))
        return t.reshape(H, FC, 128, D).transpose(0, 2, 1, 3)

    kt_full = np.stack([swz_key(K8.astype(np.float32)),
                        swz_key(Kr.astype(np.float32))], axis=2)
    kt_full = kt_full.reshape(H, 128, 2 * FC * D).astype(e4)

    # per-(batch-group, bp) compacted bank streams
    bkt_cols = sum(2 * FC * 2 * lp for lp in lps)
    gb_data = []
    for gb in range(GB):
        bkt_c = np.zeros((128, bkt_cols), dtype=e4)
        bkn_rows = []
        sb_c = []
        col = 0
        for j in range(NBP):
            lp, lh = lps[j], lhs_[j]
            bc = np.zeros((2, lp, F), dtype=np.float32)
            bias = np.zeros((2, lp), dtype=np.float32)
            for b2 in range(2):
                bsrc = perm[gb * BPC + j * 2 + b2]
                idx = np.nonzero(mask[bsrc])[0]
                bc[b2, :len(idx)] = bank[bsrc, idx]
                bias[b2, len(idx):] = -10000.0
            # bankT swizzle: [2, lp, F] -> [128(f), s, FC, 2, lp]
            t = np.ascontiguousarray(bc.transpose(0, 2, 1))     # [2, F, lp]
            t = t.reshape(2, FC, 128, lp).transpose(2, 1, 0, 3)  # [128,FC,2,lp]
            ts = t * SB
            t8 = ts.astype(e4)
            tr = (ts - t8.astype(np.float32)).astype(e4)
            blk = np.stack([tr, t8.astype(e4)], axis=1).reshape(
                128, 2 * FC * 2 * lp)
            w = 2 * FC * 2 * lp
            bkt_c[:, col:col + w] = blk
            col += w
            # bkn rows [lh, (b2, lc, F)]
            bkn_rows.append(bc.reshape(2, 2, lh, F).transpose(2, 0, 1, 3)
                            .reshape(lh, 4 * F))
            sb_c.append(bias.reshape(4 * lh))
        xt_gb = np.ascontiguousarray(
            xs[gb * BPC:(gb + 1) * BPC].T.reshape(EC, 128, BPC)
            .transpose(1, 0, 2).reshape(128, EC * BPC)).astype(np.float16)
        gb_data.append({
            "xt": xt_gb,
            "bkt": bkt_c,
            "bkn": np.ascontiguousarray(np.concatenate(bkn_rows, axis=0))
            .astype(ml_dtypes.bfloat16),
            "sbias": np.concatenate(sb_c)[None, :].astype(np.float32),
        })

    in_maps = []
    for c in range(NCORES):
        gb, gh = c // GH, c % GH
        m = dict(gb_data[gb])
        m["qt"] = qt_full[gh * HL:(gh + 1) * HL]
        m["kt"] = kt_full[gh * HL:(gh + 1) * HL]
        in_maps.append(m)
    return in_maps


_NC_CACHE = {}


def kernel(x, bank, mask, Query, Key):
    mask = np.asarray(mask)
    perm, lps = _slot_plan(mask)
    if lps not in _NC_CACHE:
        _NC_CACHE[lps] = _build_program(lps)
    nc = _NC_CACHE[lps]
    in_maps = _host_prep(x, bank, mask, Query, Key, perm, lps)

    trace = os.environ.get("KERNEL_TRACE", "0") == "1"
    res = bass_utils.run_bass_kernel_spmd(nc, in_maps,
                                          core_ids=list(range(NCORES)),
                                          trace=trace)
    if trace:
        print("exec_time_ns:", res.exec_time_ns,
              "mean:", res.mean_exec_time_ns,
              "core:", res.max_exec_time_core_id)
    full = np.empty((B, H, F), dtype=np.float32)
    for c, r in enumerate(res.results):
        gb, gh = c // GH, c % GH
        a = r["out"].astype(np.float32).reshape(128, NBP, 2, FC, HL)
        # [p, j, b2, fc, h] -> [(j b2), h, (fc p)]
        full[perm[gb * BPC:(gb + 1) * BPC], gh * HL:(gh + 1) * HL] = (
            a.transpose(1, 2, 4, 3, 0).reshape(BPC, HL, F))
    return np.ascontiguousarray(full)


# revision 20
# speedup vs baseline: 1.0422x; 1.0040x over previous
"""Trainium2 Bass kernel for nn_AttentionModule (sparse_attention).

Reference computation:
  q = tanh(einsum('hde,be->hbd', Query, x))          H=8 D=256 E=1536
  k = tanh(einsum('hdf,blf->hbld', Key, bank))       B=64 L=256 F=768
  s = einsum('hbld,hbd->hbl', k, q)  masked softmax over l
  out = LeakyReLU_0.4(einsum('hbl,blf->bhf', attn, bank))

Strategy (hybrid shard: 4 batch-groups x 2 head-groups over 8 cores):
 * Each core owns 16 b's (8 sorted pairs) and 4 heads.  This halves the
   replicated Query/Key DMA vs pure batch-parallel (the serial DMA wire,
   ~0.36 MB/us, is the binding resource): per-core input drops from
   12.9 MB to ~11.3 MB, and the 3.15 MB Query stream lands by ~18 us so
   the score/softmax pipeline never waits on it.
 * Mask compaction: the 0/1 mask keeps <=~152 of 256 bank columns per b;
   the host gathers unmasked columns, sorts b's by count, and pads each
   pair-slot to the max of its 8 ranked b's.  Padding columns get a -1e4
   additive score bias (exp -> 0) via an extra matmul.
 * The dominant k-matmul runs as error-compensated fp8 (e4m3): with
   Key*64 ~ K8 + Kr and bank*16 ~ B8 + Br, kraw = K8B8 + K8Br + KrB8.
   All three terms share one power-of-two scale, folded into the tanh
   eviction's `scale`.  Each product pair is a DoubleRow matmul.
 * k psums for both 128-row d-chunks live in one two-bank PSUM tile so a
   single tanh eviction drains 2*lpp columns (halves Act instruction
   count; per-instruction PSUM access overhead is ~185ns).
 * Narrow dims (b-pair 2, heads 4) ride in the moving dimension: q,
   score, and emb matmuls cost ap_size 16/1/4 per instruction.
 * score/softmax/emb/out for bp0..6 run as hooks inside the k4..k7 head
   loops; bp7's score matmuls ride one head behind k7's evictions, so
   after the last eviction only exp/softmax/emb/out-DMA remain.
 * Softmax skips max-subtraction (|score| < 40, exp in bf16 is safe);
   1/z is broadcast to [f, h] via a ones-matrix matmul and applied with
   LeakyReLU via one DVE multiply.  One small out-DMA per bp.
"""

import os
import numpy as np
import ml_dtypes

import concourse.bass as bass  # noqa: F401
import concourse.mybir as mybir
import concourse.tile as tile
from concourse import bacc, bass_utils

F32 = mybir.dt.float32
F16 = mybir.dt.float16
BF16 = mybir.dt.bfloat16
FP8 = mybir.dt.float8e4
AF = mybir.ActivationFunctionType
DR = mybir.MatmulPerfMode.DoubleRow

H, D, E, F = 8, 256, 1536, 768
B, L = 64, 256
NCORES = 8
GB, GH = 4, 2              # batch groups x head groups
HL = H // GH               # 4 local heads
BPC = B // GB              # 16 b's per core
NBP = BPC // 2             # 8 b-pairs per core
EC, FC, DC = E // 128, F // 128, D // 128   # 12, 6, 2
# Per-bp padded unmasked-column counts (host sorts 64 b's by count; bp_j
# takes ranks [8j, 8j+8)).  Defaults match the fixed harness input.
LPS_DEFAULT = (152, 136, 132, 130, 128, 126, 124, 120)
SK, SB = 64.0, 16.0        # fp8 pre-scales for Key / bank (powers of two)


def _build_program(lps=LPS_DEFAULT):
    assert all(lp % 2 == 0 for lp in lps)
    lhs_ = [lp // 2 for lp in lps]     # l-chunks: two per b
    lpps = [2 * lp for lp in lps]      # (b2, l') columns per (h, dc) group
    kt_cols = 2 * FC * D               # per-h Key cols ([K8, Kr] streams)
    bkt_cols = [2 * FC * w for w in lpps]     # per-bp bankT cols
    bkt_off = np.cumsum([0] + bkt_cols).tolist()
    bkn_off = np.cumsum([0] + [lh for lh in lhs_]).tolist()
    sb_off = np.cumsum([0] + [4 * lh for lh in lhs_]).tolist()
    tanh_scale = 1.0 / (SK * SB)

    nc = bacc.Bacc("TRN2", target_bir_lowering=False, debug=False,
                   enable_asserts=False, num_devices=NCORES)
    qt = nc.dram_tensor("qt", [HL, 128, EC * D], F16, kind="ExternalInput").ap()
    xt = nc.dram_tensor("xt", [128, EC * BPC], F16, kind="ExternalInput").ap()
    kt = nc.dram_tensor("kt", [HL, 128, kt_cols], FP8, kind="ExternalInput").ap()
    bkt = nc.dram_tensor("bkt", [128, bkt_off[-1]], FP8, kind="ExternalInput").ap()
    bkn = nc.dram_tensor("bkn", [bkn_off[-1], 4 * F], BF16, kind="ExternalInput").ap()
    sbias = nc.dram_tensor("sbias", [1, sb_off[-1]], F32, kind="ExternalInput").ap()
    out = nc.dram_tensor("out", [128, NBP * 2 * FC * HL], F16,
                         kind="ExternalOutput").ap()

    with tile.TileContext(nc) as tc:
        with tc.tile_pool(name="const", bufs=1) as cpool, \
             tc.tile_pool(name="weights", bufs=1) as wpool, \
             tc.tile_pool(name="bktp", bufs=1) as bpool, \
             tc.tile_pool(name="bknp", bufs=1) as npool, \
             tc.tile_pool(name="ksb", bufs=1) as kpool, \
             tc.tile_pool(name="small", bufs=4) as spool, \
             tc.tile_pool(name="outp", bufs=NBP) as opool, \
             tc.tile_pool(name="psK", bufs=2, space="PSUM") as psK, \
             tc.tile_pool(name="psS", bufs=2, space="PSUM") as psS, \
             tc.tile_pool(name="psT", bufs=2, space="PSUM") as psT:

            # ---------------- SBUF tiles ----------------------------------
            xt_sb = cpool.tile([128, EC * BPC], F16)
            kt_sb = [wpool.tile([128, kt_cols], FP8, name=f"kt{h}", tag=f"kt{h}")
                     for h in range(HL)]
            qt_sb = [wpool.tile([128, EC * D], F16, name=f"qt{h}", tag=f"qt{h}")
                     for h in range(HL)]
            bkt_t = [bpool.tile([128, bkt_cols[bp]], FP8,
                                name=f"bkt{bp}", tag=f"bkt{bp}")
                     for bp in range(NBP)]
            bkn_t = [npool.tile([lhs_[bp], 4 * F], BF16,
                                name=f"bkn{bp}", tag=f"bkn{bp}")
                     for bp in range(NBP)]
            sb_sb = cpool.tile([1, sb_off[-1]], F32)
            onesb = cpool.tile([1, BPC], F32)
            ones_mat = cpool.tile([lhs_[0], 128], BF16)
            q_sb = cpool.tile([128, 128], F16)

            # ---------------- DMA: priority order -------------------------
            def dma_bkt(bp, s=None):
                o = bkt_off[bp]
                w = bkt_cols[bp]
                if s is None:
                    nc.sync.dma_start(bkt_t[bp][:], bkt[:, o:o + w])
                else:
                    h2 = w // 2
                    nc.sync.dma_start(bkt_t[bp][:, s * h2:(s + 1) * h2],
                                      bkt[:, o + s * h2:o + (s + 1) * h2])

            # kt0/kt1 K8-halves and bkt0/1 B8-halves first so the warm-phase
            # T1 matmuls (k0's h0/h1 lead, then k1 joins) start early; the
            # residual (Kr/Br) streams follow for the cross terms
            hk = kt_cols // 2
            nc.sync.dma_start(kt_sb[0][:, 0:hk], kt[0, :, 0:hk])
            dma_bkt(0, 1)
            dma_bkt(1, 1)
            nc.sync.dma_start(kt_sb[1][:, 0:hk], kt[1, :, 0:hk])
            nc.sync.dma_start(kt_sb[0][:, hk:2 * hk], kt[0, :, hk:2 * hk])
            dma_bkt(0, 0)
            dma_bkt(1, 0)
            nc.sync.dma_start(kt_sb[1][:, hk:2 * hk], kt[1, :, hk:2 * hk])
            nc.sync.dma_start(xt_sb[:], xt)
            nc.vector.memset(onesb[:], 1.0)
            nc.vector.memset(ones_mat[:], 1.0)
            nc.sync.dma_start(kt_sb[2][:], kt[2])
            dma_bkt(2)
            nc.sync.dma_start(kt_sb[3][:], kt[3])
            dma_bkt(3)
            nc.sync.dma_start(qt_sb[0][:], qt[0])
            dma_bkt(4)
            nc.sync.dma_start(qt_sb[1][:], qt[1])
            dma_bkt(5)
            nc.sync.dma_start(qt_sb[2][:], qt[2])
            nc.sync.dma_start(qt_sb[3][:], qt[3])
            nc.sync.dma_start(sb_sb[:], sbias)
            dma_bkt(6)
            dma_bkt(7)
            for bp in range(NBP):
                nc.sync.dma_start(bkn_t[bp][:],
                                  bkn[bkn_off[bp]:bkn_off[bp + 1]])

            # ---------------- score / softmax / emb helpers ---------------
            k_sb = {}

            # Dependency tracking is TILE-granular: any write of a PSUM tile
            # serializes against all prior reads of that tile.  So every bp
            # gets its OWN [128, 512] score/z/emb tile (bp0..3 rotate psS,
            # bp4..7 rotate psT) and the 8 softmax chains pipeline freely.
            sc_ps = {}

            def score_tile(bp):
                if bp not in sc_ps:
                    pool = psT if bp >= 4 else psS
                    sc_ps[bp] = pool.tile([128, 512], F32,
                                          name="mix", tag="mix")
                return sc_ps[bp]

            def score_mms(bp, h):
                """Score matmuls for one head: 8 ap-1 matmuls (+4 bias
                matmuls at h==0).  Accumulates into ps[0:lh, 0:4*HL]."""
                ps = score_tile(bp)
                lh, lp, lpp = lhs_[bp], lps[bp], lpps[bp]
                for b2 in range(2):
                    for lc in range(2):
                        col = (b2 * 2 + lc) * HL
                        if h == 0:
                            boff = sb_off[bp] + (b2 * 2 + lc) * lh
                            nc.tensor.matmul(ps[0:lh, col:col + HL],
                                             sb_sb[:, boff:boff + lh],
                                             onesb[:, 0:HL],
                                             start=True, stop=False)
                        for dc in range(DC):
                            nc.tensor.matmul(
                                ps[0:lh, col + h:col + h + 1],
                                k_sb[(bp, h)][:, dc * lpp + b2 * lp +
                                              lc * lh:dc * lpp + b2 * lp +
                                              lc * lh + lh],
                                q_sb[:, (h * DC + dc) * BPC + bp * 2 + b2:
                                     (h * DC + dc) * BPC + bp * 2 + b2 + 1],
                                start=False,
                                stop=(h == HL - 1 and dc == DC - 1))

            exps = {}

            def score_exp(bp):
                ps = sc_ps[bp]
                lh = lhs_[bp]
                exp_t = spool.tile([lhs_[0], 4 * HL], BF16,
                                   name="exp", tag="exp")
                nc.scalar.activation(exp_t[0:lh, :], ps[0:lh, 0:4 * HL],
                                     AF.Exp)
                exps[bp] = (ps, exp_t)

            def score_full(bp):
                for h in range(HL):
                    score_mms(bp, h)
                score_exp(bp)

            def rest_part(bp):
                ps, exp_t = exps.pop(bp)
                sc_ps.pop(bp)
                lh = lhs_[bp]
                # z[b2, h] (cols 32:40): the ones-MATRIX lhsT emits column
                # sums broadcast across all 128 partitions, so no separate
                # broadcast step is needed.  Both matmul groups (z, emb) are
                # issued back-to-back on PE; recip and prelu are then both
                # READS of ps and run concurrently (tile-granular deps).
                ev = exp_t[0:lh, :].rearrange("p (b2 lc h) -> p b2 lc h",
                                              b2=2, lc=2)
                for lc in range(2):
                    nc.tensor.matmul(ps[:, 32:32 + 2 * HL],
                                     ones_mat[0:lh, :], ev[:, :, lc],
                                     start=(lc == 0), stop=(lc == 1))
                # emb[f, (b2, fc, h)] (cols 192:240)
                for b2 in range(2):
                    for fc in range(FC):
                        col = 192 + (b2 * FC + fc) * HL
                        for lc in range(2):
                            nc.tensor.matmul(
                                ps[:, col:col + HL],
                                bkn_t[bp][:, (b2 * 2 + lc) * F + fc * 128:
                                          (b2 * 2 + lc) * F + fc * 128 + 128],
                                exp_t[0:lh, (b2 * 2 + lc) * HL:
                                      (b2 * 2 + lc + 1) * HL],
                                start=(lc == 0), stop=(lc == 1))
                rz = spool.tile([128, 2 * HL], F32, name="rz", tag="rz")
                nc.vector.reciprocal(rz[:], ps[:, 32:32 + 2 * HL])
                o1 = spool.tile([128, 2 * FC * HL], F32, name="o1", tag="o1")
                w = 2 * FC * HL
                # m = emb * (1/z); then LeakyReLU entirely on the (idle)
                # DVE as one fused op max(0.4*m, m) -- keeping Act free for
                # the k-phase tanh evictions
                vb = rz[:].rearrange(
                    "p (b2 one h) -> p b2 one h", b2=2,
                    one=1).broadcast_to([128, 2, FC, HL])
                nc.vector.tensor_mul(
                    o1[:].rearrange("p (b2 fc h) -> p b2 fc h", b2=2, fc=FC),
                    ps[:, 192:192 + w].rearrange("p (b2 fc h) -> p b2 fc h",
                                                 b2=2, fc=FC),
                    vb)
                # per-bp SBUF out tile: sharing one o2 tile would serialize
                # the muls behind earlier bps' out-DMA reads
                o2 = opool.tile([128, w], F16, name=f"o2_{bp}",
                                tag=f"o2_{bp}")
                nc.vector.scalar_tensor_tensor(
                    o2[:], o1[:], 0.4, o1[:],
                    op0=mybir.AluOpType.mult, op1=mybir.AluOpType.max)
                # per-bp out-DMA: earlier bps stream out mid-kernel, only
                # bp7's small transfer sits on the tail
                nc.sync.dma_start(out[:, bp * w:(bp + 1) * w], o2[:])

            # ---------------- k = tanh(Key @ bankT) -----------------------
            def k_phase(bps, warm=False, hooks=None, tail_bp=None):
                def t1_mms(bp, h, ps2):
                    lpp = lpps[bp]
                    vb = bkt_t[bp][:].rearrange("p (s ft c) -> p s ft c",
                                                s=2, ft=FC)
                    vk = kt_sb[h][:].rearrange("p (s ft d) -> p s ft d",
                                               s=2, ft=FC)
                    for dc in range(DC):
                        g = ps2[:, dc * 512:dc * 512 + lpp]
                        for p in range(FC // 2):
                            nc.tensor.matmul(
                                g,
                                vk[:, 0, 2 * p:2 * p + 2,
                                   dc * 128:(dc + 1) * 128],
                                vb[:, 1, 2 * p:2 * p + 2],
                                start=(p == 0), stop=False, perf_mode=DR)

                def cross_evict(bp, h, ps2):
                    lpp = lpps[bp]
                    vb = bkt_t[bp][:].rearrange("p (s ft c) -> p s ft c",
                                                s=2, ft=FC)
                    vk = kt_sb[h][:].rearrange("p (s ft d) -> p s ft d",
                                               s=2, ft=FC)
                    kt_out = kpool.tile([128, 2 * lpp], F16,
                                        name=f"k{bp}_{h}", tag=f"k{bp}_{h}")
                    for dc in range(DC):
                        g = ps2[:, dc * 512:dc * 512 + lpp]
                        # cross terms: K8.Br + Kr.B8 per f-tile
                        for ft in range(FC):
                            nc.tensor.matmul(
                                g, vk[:, :, ft, dc * 128:(dc + 1) * 128],
                                vb[:, :, ft],
                                start=False, stop=(ft == FC - 1),
                                perf_mode=DR)
                    # one eviction drains both d-chunks: the [128, 2, lpp]
                    # AP walks the two PSUM banks of the ps2 tile
                    nc.scalar.activation(
                        kt_out[:].rearrange("p (dc c) -> p dc c", dc=2),
                        ps2[:].rearrange("p (dc c) -> p dc c",
                                         dc=2)[:, :, 0:lpp],
                        AF.Tanh, scale=tanh_scale)
                    k_sb[(bp, h)] = kt_out

                def new_ps():
                    return psK.tile([128, 1024], F32, name="psk", tag="psk")

                start_h = 0
                if warm:
                    # bp0's first two heads: T1s lead (they need only the
                    # K8/B8 slices), crosses follow, then later bps join
                    tiles = {}
                    for h in range(2):
                        tiles[h] = new_ps()
                        t1_mms(bps[0], h, tiles[h])
                    for h in range(2):
                        cross_evict(bps[0], h, tiles.pop(h))
                    for bp in bps[1:]:
                        for h in range(2):
                            ps2 = new_ps()
                            t1_mms(bp, h, ps2)
                            cross_evict(bp, h, ps2)
                    start_h = 2
                for h in range(start_h, HL):
                    for bp in bps:
                        ps2 = new_ps()
                        t1_mms(bp, h, ps2)
                        cross_evict(bp, h, ps2)
                        if tail_bp is not None and h >= 1:
                            # the phase's own bp's score rides one head
                            # behind its evictions (the h-1 eviction ended
                            # while this head's 18 matmuls ran)
                            score_mms(tail_bp, h - 1)
                    if hooks and h in hooks:
                        for fn in hooks[h]:
                            fn()

            # warm phase: k0+k1 interleaved per head (2.3us of PE work per
            # kt[h] arrival so the lead-in is never DMA-starved)
            k_phase([0, 1], warm=True)
            k_phase([2])
            k_phase([3])

            # ---------------- q = tanh(Query @ x) -------------------------
            # qt (3.15 MB) has streamed in behind the k inputs by now.
            # The q psum borrows a psS buffer (freed at the tanh eviction,
            # before bp1's score needs it).
            psq = psS.tile([128, 512], F32, name="mix", tag="mix")
            for h in range(HL):
                vq = qt_sb[h][:].rearrange("p (ec d) -> p ec d", ec=EC)
                for dc in range(DC):
                    g = psq[:, (h * DC + dc) * BPC:(h * DC + dc + 1) * BPC]
                    for ec in range(EC):
                        nc.tensor.matmul(
                            g, vq[:, ec, dc * 128:(dc + 1) * 128],
                            xt_sb[:, ec * BPC:(ec + 1) * BPC],
                            start=(ec == 0), stop=(ec == EC - 1))
            nc.scalar.activation(q_sb[:], psq[:, 0:128], AF.Tanh)

            # ---------------- k4..k7 with score/softmax hooks -------------
            # bp4..7's own score matmuls ride one head behind their own
            # phase's evictions (tail_bp); fin(bp) = last head + exp at the
            # next phase's start.  bp0..3 score/softmax spread over k4/k5's
            # slots, rests two slots after their exp.
            def fin(bp):
                score_mms(bp, HL - 1)
                score_exp(bp)

            k_phase([4], tail_bp=4,
                    hooks={0: [lambda: score_full(0)],
                           1: [lambda: score_full(1)],
                           2: [lambda: rest_part(0)],
                           3: [lambda: rest_part(1)]})
            k_phase([5], tail_bp=5,
                    hooks={0: [lambda: fin(4), lambda: score_full(2)],
                           1: [lambda: score_full(3)],
                           2: [lambda: rest_part(2), lambda: rest_part(4)],
                           3: [lambda: rest_part(3)]})
            k_phase([6], tail_bp=6,
                    hooks={0: [lambda: fin(5)],
                           2: [lambda: rest_part(5)]})
            k_phase([7], tail_bp=7,
                    hooks={0: [lambda: fin(6)],
                           2: [lambda: rest_part(6)]})
            # tail: bp7's last head, exp, softmax/emb/out
            fin(7)
            rest_part(7)

    nc.finalize()
    return nc


def _slot_plan(mask):
    """Sort b's by unmasked count (desc); bp_j takes ranks [8j, 8j+8).
    Returns (perm, lps): perm[slot] = original b, slot = gb*BPC + j*2 + b2."""
    counts = mask.sum(axis=1)
    order = np.argsort(-counts, kind="stable")
    perm = np.empty(B, dtype=np.int64)
    for j in range(NBP):
        grp = order[8 * j:8 * (j + 1)]
        for gb in range(GB):
            perm[gb * BPC + j * 2] = grp[2 * gb]
            perm[gb * BPC + j * 2 + 1] = grp[2 * gb + 1]
    lps = tuple(max(int(2 * ((counts[order[8 * j]] + 1) // 2)), 8)
                for j in range(NBP))
    return perm, lps


def _host_prep(x, bank, mask, Query, Key, perm, lps):
    x = np.asarray(x, dtype=np.float32)
    bank = np.asarray(bank, dtype=np.float32)
    mask = np.asarray(mask)
    Query = np.asarray(Query, dtype=np.float32)
    Key = np.asarray(Key, dtype=np.float32)
    e4 = ml_dtypes.float8_e4m3
    lhs_ = [lp // 2 for lp in lps]

    # q path: f16, host-transposed; per head-group slice
    xs = x[perm]
    qt_full = np.ascontiguousarray(Query.transpose(0, 2, 1)).reshape(
        H, EC, 128, D).transpose(0, 2, 1, 3).reshape(H, 128, EC * D)
    qt_full = qt_full.astype(np.float16)

    Ks = Key * SK
    K8 = Ks.astype(e4)
    Kr = (Ks - K8.astype(np.float32)).astype(e4)

    def swz_key(Kt):  # [H, D, F] -> [H, 128(f), FC, D]
        t = np.ascontiguousarray(Kt.transpose(0, 2, 1))
        return t.reshape(H, FC, 128, D).transpose(0, 2, 1, 3)

    kt_full = np.stack([swz_key(K8.astype(np.float32)),
                        swz_key(Kr.astype(np.float32))], axis=2)
    kt_full = kt_full.reshape(H, 128, 2 * FC * D).astype(e4)

    # per-(batch-group, bp) compacted bank streams
    bkt_cols = sum(2 * FC * 2 * lp for lp in lps)
    gb_data = []
    for gb in range(GB):
        bkt_c = np.zeros((128, bkt_cols), dtype=e4)
        bkn_rows = []
        sb_c = []
        col = 0
        for j in range(NBP):
            lp, lh = lps[j], lhs_[j]
            bc = np.zeros((2, lp, F), dtype=np.float32)
            bias = np.zeros((2, lp), dtype=np.float32)
            for b2 in range(2):
                bsrc = perm[gb * BPC + j * 2 + b2]
                idx = np.nonzero(mask[bsrc])[0]
                bc[b2, :len(idx)] = bank[bsrc, idx]
                bias[b2, len(idx):] = -10000.0
            # bankT swizzle: [2, lp, F] -> [128(f), s, FC, 2, lp]
            t = np.ascontiguousarray(bc.transpose(0, 2, 1))     # [2, F, lp]
            t = t.reshape(2, FC, 128, lp).transpose(2, 1, 0, 3)  # [128,FC,2,lp]
            ts = t * SB
            t8 = ts.astype(e4)
            tr = (ts - t8.astype(np.float32)).astype(e4)
            blk = np.stack([tr, t8.astype(e4)], axis=1).reshape(
                128, 2 * FC * 2 * lp)
            w = 2 * FC * 2 * lp
            bkt_c[:, col:col + w] = blk
            col += w
            # bkn rows [lh, (b2, lc, F)]
            bkn_rows.append(bc.reshape(2, 2, lh, F).transpose(2, 0, 1, 3)
                            .reshape(lh, 4 * F))
            sb_c.append(bias.reshape(4 * lh))
        xt_gb = np.ascontiguousarray(
            xs[gb * BPC:(gb + 1) * BPC].T.reshape(EC, 128, BPC)
            .transpose(1, 0, 2).reshape(128, EC * BPC)).astype(np.float16)
        gb_data.append({
            "xt": xt_gb,
            "bkt": bkt_c,
            "bkn": np.ascontiguousarray(np.concatenate(bkn_rows, axis=0))
            .astype(ml_dtypes.bfloat16),
            "sbias": np.concatenate(sb_c)[None, :].astype(np.float32),
        })

    in_maps = []
    for c in range(NCORES):
        gb, gh = c // GH, c % GH
        m = dict(gb_data[gb])
        m["qt"] = qt_full[gh * HL:(gh + 1) * HL]
        m["kt"] = kt_full[gh * HL:(gh + 1) * HL]
        in_maps.append(m)
    return in_maps


_NC_CACHE = {}


def kernel(x, bank, mask, Query, Key):
    mask = np.asarray(mask)
    perm, lps = _slot_plan(mask)
    if lps not in _NC_CACHE:
        _NC_CACHE[lps] = _build_program(lps)
    nc = _NC_CACHE[lps]
    in_maps = _host_prep(x, bank, mask, Query, Key, perm, lps)

    trace = os.environ.get("KERNEL_TRACE", "0") == "1"
    res = bass_utils.run_bass_kernel_spmd(nc, in_maps,
                                          core_ids=list(range(NCORES)),
                                          trace=trace)
    if trace:
        print("exec_time_ns:", res.exec_time_ns,
              "mean:", res.mean_exec_time_ns,
              "core:", res.max_exec_time_core_id)
    full = np.empty((B, H, F), dtype=np.float32)
    for c, r in enumerate(res.results):
        gb, gh = c // GH, c % GH
        a = r["out"].astype(np.float32).reshape(128, NBP, 2, FC, HL)
        # [p, j, b2, fc, h] -> [(j b2), h, (fc p)]
        full[perm[gb * BPC:(gb + 1) * BPC], gh * HL:(gh + 1) * HL] = (
            a.transpose(1, 2, 4, 3, 0).reshape(BPC, HL, F))
    return np.ascontiguousarray(full)


# revision 23
# speedup vs baseline: 1.0572x; 1.0143x over previous
"""Trainium2 Bass kernel for nn_AttentionModule (sparse_attention).

Reference computation:
  q = tanh(einsum('hde,be->hbd', Query, x))          H=8 D=256 E=1536
  k = tanh(einsum('hdf,blf->hbld', Key, bank))       B=64 L=256 F=768
  s = einsum('hbld,hbd->hbl', k, q)  masked softmax over l
  out = LeakyReLU_0.4(einsum('hbl,blf->bhf', attn, bank))

Strategy (hybrid shard: 4 batch-groups x 2 head-groups over 8 cores):
 * Each core owns 16 b's (8 sorted pairs) and 4 heads.  This halves the
   replicated Query/Key DMA vs pure batch-parallel (the serial DMA wire,
   ~0.36 MB/us, is the binding resource): per-core input drops from
   12.9 MB to ~11.3 MB, and the 3.15 MB Query stream lands by ~18 us so
   the score/softmax pipeline never waits on it.
 * Mask compaction: the 0/1 mask keeps <=~152 of 256 bank columns per b;
   the host gathers unmasked columns, sorts b's by count, and pads each
   pair-slot to the max of its 8 ranked b's.  Padding columns get a -1e4
   additive score bias (exp -> 0) via an extra matmul.
 * The dominant k-matmul runs as error-compensated fp8 (e4m3): with
   Key*64 ~ K8 + Kr and bank*16 ~ B8 + Br, kraw = K8B8 + K8Br + KrB8.
   All three terms share one power-of-two scale, folded into the tanh
   eviction's `scale`.  Each product pair is a DoubleRow matmul.
 * k psums for both 128-row d-chunks live in one two-bank PSUM tile so a
   single tanh eviction drains 2*lpp columns (halves Act instruction
   count; per-instruction PSUM access overhead is ~185ns).
 * Narrow dims (b-pair 2, heads 4) ride in the moving dimension: q,
   score, and emb matmuls cost ap_size 16/1/4 per instruction.
 * score/softmax/emb/out for bp0..6 run as hooks inside the k4..k7 head
   loops; bp7's score matmuls ride one head behind k7's evictions, so
   after the last eviction only exp/softmax/emb/out-DMA remain.
 * Softmax skips max-subtraction (|score| < 40, exp in bf16 is safe);
   1/z is broadcast to [f, h] via a ones-matrix matmul and applied with
   LeakyReLU via one DVE multiply.  One small out-DMA per bp.
"""

import os
import numpy as np
import ml_dtypes

import concourse.bass as bass  # noqa: F401
import concourse.mybir as mybir
import concourse.tile as tile
from concourse import bacc, bass_utils

F32 = mybir.dt.float32
F16 = mybir.dt.float16
BF16 = mybir.dt.bfloat16
FP8 = mybir.dt.float8e4
AF = mybir.ActivationFunctionType
DR = mybir.MatmulPerfMode.DoubleRow

H, D, E, F = 8, 256, 1536, 768
B, L = 64, 256
NCORES = 8
GB, GH = 4, 2              # batch groups x head groups
HL = H // GH               # 4 local heads
BPC = B // GB              # 16 b's per core
NBP = BPC // 2             # 8 b-pairs per core
EC, FC, DC = E // 128, F // 128, D // 128   # 12, 6, 2
# Per-bp padded unmasked-column counts (host sorts 64 b's by count; bp_j
# takes ranks [8j, 8j+8)).  Defaults match the fixed harness input.
LPS_DEFAULT = (152, 136, 132, 130, 128, 126, 124, 120)
SK, SB = 64.0, 16.0        # fp8 pre-scales for Key / bank (powers of two)


def _build_program(lps=LPS_DEFAULT):
    assert all(lp % 2 == 0 for lp in lps)
    lhs_ = [lp // 2 for lp in lps]     # l-chunks: two per b
    lpps = [2 * lp for lp in lps]      # (b2, l') columns per (h, dc) group
    kt_cols = 2 * FC * D               # per-h Key cols ([K8, Kr] streams)
    bkt_cols = [2 * FC * w for w in lpps]     # per-bp bankT cols
    bkt_off = np.cumsum([0] + bkt_cols).tolist()
    bkn_off = np.cumsum([0] + [lh for lh in lhs_]).tolist()
    sb_off = np.cumsum([0] + [4 * lh for lh in lhs_]).tolist()
    tanh_scale = 1.0 / (SK * SB)

    nc = bacc.Bacc("TRN2", target_bir_lowering=False, debug=False,
                   enable_asserts=False, num_devices=NCORES)
    qt = nc.dram_tensor("qt", [HL, 128, EC * D], F16, kind="ExternalInput").ap()
    xt = nc.dram_tensor("xt", [128, EC * BPC], F16, kind="ExternalInput").ap()
    kt = nc.dram_tensor("kt", [HL, 128, kt_cols], FP8, kind="ExternalInput").ap()
    bkt = nc.dram_tensor("bkt", [128, bkt_off[-1]], FP8, kind="ExternalInput").ap()
    bkn = nc.dram_tensor("bkn", [bkn_off[-1], 4 * F], BF16, kind="ExternalInput").ap()
    sbias = nc.dram_tensor("sbias", [1, sb_off[-1]], F32, kind="ExternalInput").ap()
    out = nc.dram_tensor("out", [128, NBP * 2 * FC * HL], F16,
                         kind="ExternalOutput").ap()

    with tile.TileContext(nc) as tc:
        with tc.tile_pool(name="const", bufs=1) as cpool, \
             tc.tile_pool(name="weights", bufs=1) as wpool, \
             tc.tile_pool(name="bktp", bufs=1) as bpool, \
             tc.tile_pool(name="bknp", bufs=1) as npool, \
             tc.tile_pool(name="ksb", bufs=1) as kpool, \
             tc.tile_pool(name="small", bufs=4) as spool, \
             tc.tile_pool(name="outp", bufs=NBP) as opool, \
             tc.tile_pool(name="psK", bufs=2, space="PSUM") as psK, \
             tc.tile_pool(name="psS", bufs=2, space="PSUM") as psS, \
             tc.tile_pool(name="psT", bufs=2, space="PSUM") as psT:

            # ---------------- SBUF tiles ----------------------------------
            xt_sb = cpool.tile([128, EC * BPC], F16)
            kt_sb = [wpool.tile([128, kt_cols], FP8, name=f"kt{h}", tag=f"kt{h}")
                     for h in range(HL)]
            qt_sb = [wpool.tile([128, EC * D], F16, name=f"qt{h}", tag=f"qt{h}")
                     for h in range(HL)]
            bkt_t = [bpool.tile([128, bkt_cols[bp]], FP8,
                                name=f"bkt{bp}", tag=f"bkt{bp}")
                     for bp in range(NBP)]
            bkn_t = [npool.tile([lhs_[bp], 4 * F], BF16,
                                name=f"bkn{bp}", tag=f"bkn{bp}")
                     for bp in range(NBP)]
            sb_sb = cpool.tile([1, sb_off[-1]], F32)
            onesb = cpool.tile([1, BPC], F32)
            ones_mat = cpool.tile([lhs_[0], 128], BF16)
            q_sb = cpool.tile([128, 128], F16)

            # ---------------- DMA: priority order -------------------------
            def dma_bkt(bp, s=None):
                o = bkt_off[bp]
                w = bkt_cols[bp]
                if s is None:
                    nc.sync.dma_start(bkt_t[bp][:], bkt[:, o:o + w])
                else:
                    h2 = w // 2
                    nc.sync.dma_start(bkt_t[bp][:, s * h2:(s + 1) * h2],
                                      bkt[:, o + s * h2:o + (s + 1) * h2])

            # kt0/kt1 K8-halves and bkt0/1 B8-halves first so the warm-phase
            # T1 matmuls (k0's h0/h1 lead, then k1 joins) start early; the
            # residual (Kr/Br) streams follow for the cross terms
            hk = kt_cols // 2
            nc.sync.dma_start(kt_sb[0][:, 0:hk], kt[0, :, 0:hk])
            dma_bkt(0, 1)
            dma_bkt(1, 1)
            nc.sync.dma_start(kt_sb[1][:, 0:hk], kt[1, :, 0:hk])
            nc.sync.dma_start(kt_sb[0][:, hk:2 * hk], kt[0, :, hk:2 * hk])
            dma_bkt(0, 0)
            dma_bkt(1, 0)
            nc.sync.dma_start(kt_sb[1][:, hk:2 * hk], kt[1, :, hk:2 * hk])
            nc.sync.dma_start(xt_sb[:], xt)
            nc.vector.memset(onesb[:], 1.0)
            nc.vector.memset(ones_mat[:], 1.0)
            nc.sync.dma_start(kt_sb[2][:], kt[2])
            dma_bkt(2)
            nc.sync.dma_start(kt_sb[3][:], kt[3])
            dma_bkt(3)
            nc.sync.dma_start(qt_sb[0][:], qt[0])
            dma_bkt(4)
            nc.sync.dma_start(qt_sb[1][:], qt[1])
            dma_bkt(5)
            nc.sync.dma_start(qt_sb[2][:], qt[2])
            nc.sync.dma_start(qt_sb[3][:], qt[3])
            nc.sync.dma_start(sb_sb[:], sbias)
            dma_bkt(6)
            dma_bkt(7)
            for bp in range(NBP):
                nc.sync.dma_start(bkn_t[bp][:],
                                  bkn[bkn_off[bp]:bkn_off[bp + 1]])

            # ---------------- score / softmax / emb helpers ---------------
            k_sb = {}

            # Dependency tracking is TILE-granular: any write of a PSUM tile
            # serializes against all prior reads of that tile.  So every bp
            # gets its OWN [128, 512] score/z/emb tile (bp0..3 rotate psS,
            # bp4..7 rotate psT) and the 8 softmax chains pipeline freely.
            sc_ps = {}

            def score_tile(bp):
                if bp not in sc_ps:
                    # alternate pools so each buffer's next use is 4
                    # score/rest pairs later (rotation never gates)
                    pool = psT if bp % 2 == 0 else psS
                    sc_ps[bp] = pool.tile([128, 512], F32,
                                          name="mix", tag="mix")
                return sc_ps[bp]

            def score_mms(bp, h):
                """Score matmuls for one head: 8 ap-1 matmuls (+4 bias
                matmuls at h==0).  Accumulates into ps[0:lh, 0:4*HL]."""
                ps = score_tile(bp)
                lh, lp, lpp = lhs_[bp], lps[bp], lpps[bp]
                for b2 in range(2):
                    for lc in range(2):
                        col = (b2 * 2 + lc) * HL
                        if h == 0:
                            boff = sb_off[bp] + (b2 * 2 + lc) * lh
                            nc.tensor.matmul(ps[0:lh, col:col + HL],
                                             sb_sb[:, boff:boff + lh],
                                             onesb[:, 0:HL],
                                             start=True, stop=False)
                        for dc in range(DC):
                            nc.tensor.matmul(
                                ps[0:lh, col + h:col + h + 1],
                                k_sb[(bp, h)][:, dc * lpp + b2 * lp +
                                              lc * lh:dc * lpp + b2 * lp +
                                              lc * lh + lh],
                                q_sb[:, (h * DC + dc) * BPC + bp * 2 + b2:
                                     (h * DC + dc) * BPC + bp * 2 + b2 + 1],
                                start=False,
                                stop=(h == HL - 1 and dc == DC - 1))

            exps = {}

            def score_exp(bp):
                ps = sc_ps[bp]
                lh = lhs_[bp]
                exp_t = spool.tile([lhs_[0], 4 * HL], BF16,
                                   name="exp", tag="exp")
                nc.scalar.activation(exp_t[0:lh, :], ps[0:lh, 0:4 * HL],
                                     AF.Exp)
                exps[bp] = (ps, exp_t)

            def score_full(bp):
                for h in range(HL):
                    score_mms(bp, h)
                score_exp(bp)

            def rest_part(bp):
                ps, exp_t = exps.pop(bp)
                sc_ps.pop(bp)
                lh = lhs_[bp]
                # z[b2, h] (cols 32:40): the ones-MATRIX lhsT emits column
                # sums broadcast across all 128 partitions, so no separate
                # broadcast step is needed.  Both matmul groups (z, emb) are
                # issued back-to-back on PE; recip and prelu are then both
                # READS of ps and run concurrently (tile-granular deps).
                ev = exp_t[0:lh, :].rearrange("p (b2 lc h) -> p b2 lc h",
                                              b2=2, lc=2)
                for lc in range(2):
                    nc.tensor.matmul(ps[:, 32:32 + 2 * HL],
                                     ones_mat[0:lh, :], ev[:, :, lc],
                                     start=(lc == 0), stop=(lc == 1))
                # emb[f, (b2, fc, h)] (cols 192:240)
                for b2 in range(2):
                    for fc in range(FC):
                        col = 192 + (b2 * FC + fc) * HL
                        for lc in range(2):
                            nc.tensor.matmul(
                                ps[:, col:col + HL],
                                bkn_t[bp][:, (b2 * 2 + lc) * F + fc * 128:
                                          (b2 * 2 + lc) * F + fc * 128 + 128],
                                exp_t[0:lh, (b2 * 2 + lc) * HL:
                                      (b2 * 2 + lc + 1) * HL],
                                start=(lc == 0), stop=(lc == 1))
                rz = spool.tile([128, 2 * HL], F32, name="rz", tag="rz")
                nc.vector.reciprocal(rz[:], ps[:, 32:32 + 2 * HL])
                o1 = spool.tile([128, 2 * FC * HL], F32, name="o1", tag="o1")
                w = 2 * FC * HL
                # m = emb * (1/z); then LeakyReLU entirely on the (idle)
                # DVE as one fused op max(0.4*m, m) -- keeping Act free for
                # the k-phase tanh evictions
                vb = rz[:].rearrange(
                    "p (b2 one h) -> p b2 one h", b2=2,
                    one=1).broadcast_to([128, 2, FC, HL])
                nc.vector.tensor_mul(
                    o1[:].rearrange("p (b2 fc h) -> p b2 fc h", b2=2, fc=FC),
                    ps[:, 192:192 + w].rearrange("p (b2 fc h) -> p b2 fc h",
                                                 b2=2, fc=FC),
                    vb)
                # per-bp SBUF out tile: sharing one o2 tile would serialize
                # the muls behind earlier bps' out-DMA reads
                o2 = opool.tile([128, w], F16, name=f"o2_{bp}",
                                tag=f"o2_{bp}")
                nc.vector.scalar_tensor_tensor(
                    o2[:], o1[:], 0.4, o1[:],
                    op0=mybir.AluOpType.mult, op1=mybir.AluOpType.max)
                # per-bp out-DMA: earlier bps stream out mid-kernel, only
                # bp7's small transfer sits on the tail
                nc.sync.dma_start(out[:, bp * w:(bp + 1) * w], o2[:])

            # ---------------- k = tanh(Key @ bankT) -----------------------
            def k_phase(bps, warm=False, hooks=None, tail_bp=None):
                def t1_mms(bp, h, ps2):
                    lpp = lpps[bp]
                    vb = bkt_t[bp][:].rearrange("p (s ft c) -> p s ft c",
                                                s=2, ft=FC)
                    vk = kt_sb[h][:].rearrange("p (s ft d) -> p s ft d",
                                               s=2, ft=FC)
                    for dc in range(DC):
                        g = ps2[:, dc * 512:dc * 512 + lpp]
                        for p in range(FC // 2):
                            nc.tensor.matmul(
                                g,
                                vk[:, 0, 2 * p:2 * p + 2,
                                   dc * 128:(dc + 1) * 128],
                                vb[:, 1, 2 * p:2 * p + 2],
                                start=(p == 0), stop=False, perf_mode=DR)

                def cross_evict(bp, h, ps2):
                    lpp = lpps[bp]
                    vb = bkt_t[bp][:].rearrange("p (s ft c) -> p s ft c",
                                                s=2, ft=FC)
                    vk = kt_sb[h][:].rearrange("p (s ft d) -> p s ft d",
                                               s=2, ft=FC)
                    kt_out = kpool.tile([128, 2 * lpp], F16,
                                        name=f"k{bp}_{h}", tag=f"k{bp}_{h}")
                    for dc in range(DC):
                        g = ps2[:, dc * 512:dc * 512 + lpp]
                        # cross terms: K8.Br + Kr.B8 per f-tile
                        for ft in range(FC):
                            nc.tensor.matmul(
                                g, vk[:, :, ft, dc * 128:(dc + 1) * 128],
                                vb[:, :, ft],
                                start=False, stop=(ft == FC - 1),
                                perf_mode=DR)
                    # one eviction drains both d-chunks: the [128, 2, lpp]
                    # AP walks the two PSUM banks of the ps2 tile
                    nc.scalar.activation(
                        kt_out[:].rearrange("p (dc c) -> p dc c", dc=2),
                        ps2[:].rearrange("p (dc c) -> p dc c",
                                         dc=2)[:, :, 0:lpp],
                        AF.Tanh, scale=tanh_scale)
                    k_sb[(bp, h)] = kt_out

                def new_ps():
                    return psK.tile([128, 1024], F32, name="psk", tag="psk")

                start_h = 0
                if warm:
                    # bp0's first two heads: T1s lead (they need only the
                    # K8/B8 slices), crosses follow, then later bps join
                    tiles = {}
                    for h in range(2):
                        tiles[h] = new_ps()
                        t1_mms(bps[0], h, tiles[h])
                    for h in range(2):
                        cross_evict(bps[0], h, tiles.pop(h))
                    for bp in bps[1:]:
                        for h in range(2):
                            ps2 = new_ps()
                            t1_mms(bp, h, ps2)
                            cross_evict(bp, h, ps2)
                    start_h = 2
                for h in range(start_h, HL):
                    for bp in bps:
                        ps2 = new_ps()
                        t1_mms(bp, h, ps2)
                        cross_evict(bp, h, ps2)
                        if tail_bp is not None and h >= 1:
                            # the phase's own bp's score rides one head
                            # behind its evictions (the h-1 eviction ended
                            # while this head's 18 matmuls ran)
                            score_mms(tail_bp, h - 1)
                    if hooks and h in hooks:
                        for fn in hooks[h]:
                            fn()

            # warm phase: k0+k1 interleaved per head (2.3us of PE work per
            # kt[h] arrival so the lead-in is never DMA-starved)
            k_phase([0, 1], warm=True)
            k_phase([2])
            k_phase([3])

            # ---------------- q = tanh(Query @ x) -------------------------
            # qt (3.15 MB) has streamed in behind the k inputs by now.
            # The q psum borrows a psS buffer (freed at the tanh eviction,
            # before bp1's score needs it).
            psq = psS.tile([128, 512], F32, name="mix", tag="mix")
            for h in range(HL):
                vq = qt_sb[h][:].rearrange("p (ec d) -> p ec d", ec=EC)
                for dc in range(DC):
                    g = psq[:, (h * DC + dc) * BPC:(h * DC + dc + 1) * BPC]
                    for ec in range(EC):
                        nc.tensor.matmul(
                            g, vq[:, ec, dc * 128:(dc + 1) * 128],
                            xt_sb[:, ec * BPC:(ec + 1) * BPC],
                            start=(ec == 0), stop=(ec == EC - 1))
            nc.scalar.activation(q_sb[:], psq[:, 0:128], AF.Tanh)

            # ---------------- k4..k7 + score/softmax pipeline -------------
            k_phase([4])
            k_phase([5])
            k_phase([6])
            # score/softmax/emb pipeline for bp0..6, issued (priority-
            # earlier) before k7's matmuls: the Tile scheduler weaves the
            # k7 stream around these small parked ops, so their chains
            # drain while the PE is still busy with k-matmuls
            pending = None
            for bp in range(NBP - 1):
                score_full(bp)
                if pending is not None:
                    rest_part(pending)
                pending = bp
            k_phase([7], hooks={1: [lambda: rest_part(NBP - 2)]})
            # tail: bp7's score/exp/softmax/emb/out
            score_full(NBP - 1)
            rest_part(NBP - 1)

    nc.finalize()
    return nc


def _slot_plan(mask):
    """Sort b's by unmasked count (desc); bp_j takes ranks [8j, 8j+8).
    Returns (perm, lps): perm[slot] = original b, slot = gb*BPC + j*2 + b2."""
    counts = mask.sum(axis=1)
    order = np.argsort(-counts, kind="stable")
    perm = np.empty(B, dtype=np.int64)
    for j in range(NBP):
        grp = order[8 * j:8 * (j + 1)]
        for gb in range(GB):
            perm[gb * BPC + j * 2] = grp[2 * gb]
            perm[gb * BPC + j * 2 + 1] = grp[2 * gb + 1]
    lps = tuple(max(int(2 * ((counts[order[8 * j]] + 1) // 2)), 8)
                for j in range(NBP))
    return perm, lps


def _host_prep(x, bank, mask, Query, Key, perm, lps):
    x = np.asarray(x, dtype=np.float32)
    bank = np.asarray(bank, dtype=np.float32)
    mask = np.asarray(mask)
    Query = np.asarray(Query, dtype=np.float32)
    Key = np.asarray(Key, dtype=np.float32)
    e4 = ml_dtypes.float8_e4m3
    lhs_ = [lp // 2 for lp in lps]

    # q path: f16, host-transposed; per head-group slice
    xs = x[perm]
    qt_full = np.ascontiguousarray(Query.transpose(0, 2, 1)).reshape(
        H, EC, 128, D).transpose(0, 2, 1, 3).reshape(H, 128, EC * D)
    qt_full = qt_full.astype(np.float16)

    Ks = Key * SK
    K8 = Ks.astype(e4)
    Kr = (Ks - K8.astype(np.float32)).astype(e4)

    def swz_key(Kt):  # [H, D, F] -> [H, 128(f), FC, D]
        t = np.ascontiguousarray(Kt.transpose(0, 2, 1))
        return t.reshape(H, FC, 128, D).transpose(0, 2, 1, 3)

    kt_full = np.stack([swz_key(K8.astype(np.float32)),
                        swz_key(Kr.astype(np.float32))], axis=2)
    kt_full = kt_full.reshape(H, 128, 2 * FC * D).astype(e4)

    # per-(batch-group, bp) compacted bank streams
    bkt_cols = sum(2 * FC * 2 * lp for lp in lps)
    gb_data = []
    for gb in range(GB):
        bkt_c = np.zeros((128, bkt_cols), dtype=e4)
        bkn_rows = []
        sb_c = []
        col = 0
        for j in range(NBP):
            lp, lh = lps[j], lhs_[j]
            bc = np.zeros((2, lp, F), dtype=np.float32)
            bias = np.zeros((2, lp), dtype=np.float32)
            for b2 in range(2):
                bsrc = perm[gb * BPC + j * 2 + b2]
                idx = np.nonzero(mask[bsrc])[0]
                bc[b2, :len(idx)] = bank[bsrc, idx]
                bias[b2, len(idx):] = -10000.0
            # bankT swizzle: [2, lp, F] -> [128(f), s, FC, 2, lp]
            t = np.ascontiguousarray(bc.transpose(0, 2, 1))     # [2, F, lp]
            t = t.reshape(2, FC, 128, lp).transpose(2, 1, 0, 3)  # [128,FC,2,lp]
            ts = t * SB
            t8 = ts.astype(e4)
            tr = (ts - t8.astype(np.float32)).astype(e4)
            blk = np.stack([tr, t8.astype(e4)], axis=1).reshape(
                128, 2 * FC * 2 * lp)
            w = 2 * FC * 2 * lp
            bkt_c[:, col:col + w] = blk
            col += w
            # bkn rows [lh, (b2, lc, F)]
            bkn_rows.append(bc.reshape(2, 2, lh, F).transpose(2, 0, 1, 3)
                            .reshape(lh, 4 * F))
            sb_c.append(bias.reshape(4 * lh))
        xt_gb = np.ascontiguousarray(
            xs[gb * BPC:(gb + 1) * BPC].T.reshape(EC, 128, BPC)
            .transpose(1, 0, 2).reshape(128, EC * BPC)).astype(np.float16)
        gb_data.append({
            "xt": xt_gb,
            "bkt": bkt_c,
            "bkn": np.ascontiguousarray(np.concatenate(bkn_rows, axis=0))
            .astype(ml_dtypes.bfloat16),
            "sbias": np.concatenate(sb_c)[None, :].astype(np.float32),
        })

    in_maps = []
    for c in range(NCORES):
        gb, gh = c // GH, c % GH
        m = dict(gb_data[gb])
        m["qt"] = qt_full[gh * HL:(gh + 1) * HL]
        m["kt"] = kt_full[gh * HL:(gh + 1) * HL]
        in_maps.append(m)
    return in_maps


_NC_CACHE = {}


def kernel(x, bank, mask, Query, Key):
    mask = np.asarray(mask)
    perm, lps = _slot_plan(mask)
    if lps not in _NC_CACHE:
        _NC_CACHE[lps] = _build_program(lps)
    nc = _NC_CACHE[lps]
    in_maps = _host_prep(x, bank, mask, Query, Key, perm, lps)

    trace = os.environ.get("KERNEL_TRACE", "0") == "1"
    res = bass_utils.run_bass_kernel_spmd(nc, in_maps,
                                          core_ids=list(range(NCORES)),
                                          trace=trace)
    if trace:
        print("exec_time_ns:", res.exec_time_ns,
              "mean:", res.mean_exec_time_ns,
              "core:", res.max_exec_time_core_id)
    full = np.empty((B, H, F), dtype=np.float32)
    for c, r in enumerate(res.results):
        gb, gh = c // GH, c % GH
        a = r["out"].astype(np.float32).reshape(128, NBP, 2, FC, HL)
        # [p, j, b2, fc, h] -> [(j b2), h, (fc p)]
        full[perm[gb * BPC:(gb + 1) * BPC], gh * HL:(gh + 1) * HL] = (
            a.transpose(1, 2, 4, 3, 0).reshape(BPC, HL, F))
    return np.ascontiguousarray(full)


# revision 45
# speedup vs baseline: 1.1478x; 1.0857x over previous
"""Trainium2 Bass kernel for nn_AttentionModule (sparse_attention).

Reference computation:
  q = tanh(einsum('hde,be->hbd', Query, x))          H=8 D=256 E=1536
  k = tanh(einsum('hdf,blf->hbld', Key, bank))       B=64 L=256 F=768
  s = einsum('hbld,hbd->hbl', k, q)  masked softmax over l
  out = LeakyReLU_0.4(einsum('hbl,blf->bhf', attn, bank))

Strategy (hybrid shard: 4 batch-groups x 2 head-groups over 8 cores):
 * Each core owns 16 b's (8 sorted pairs) and 4 heads.  This halves the
   replicated Query/Key DMA vs pure batch-parallel (the serial DMA wire,
   ~0.36 MB/us, binds): per-core input drops 12.9 -> ~11.3 MB, and the
   3.15 MB Query stream lands by ~17 us, so the score/softmax pipeline
   never waits on it.
 * Mask compaction: the 0/1 mask keeps <=~152 of 256 bank columns per b;
   the host gathers unmasked columns, sorts b's by count, pads each
   pair-slot to the max of its 8 ranked b's.  Padding columns get a -1e4
   additive score bias (exp -> 0) via an extra matmul.
 * The dominant k-matmul runs as error-compensated fp8 (e4m3): with
   Key*64 ~ K8 + Kr and bank*16 ~ B8 + Br, kraw = K8B8 + K8Br + KrB8
   (the fp8*fp8 residual cross term is negligible).  All three terms
   share one power-of-two scale, folded into the tanh eviction's
   `scale`.  Each product pair runs as a DoubleRow matmul.
 * Narrow dims ride in the moving dimension: q, score, and emb matmuls
   cost ap_size 16/1/4 per instruction instead of 256-512.
 * score/softmax/emb runs as FOUR quad-chains (4 b's = 2 bp-pairs per
   PSUM tile, rows padded to the quad max): same chain count as the
   well-pipelined batch-parallel version, half the Query bytes.  The
   last two k phases run joint (both bps per head) so quad3's inputs
   finish early; quad2's softmax rides inside that phase.
 * Softmax skips max-subtraction (|score| < 40, exp in bf16 is safe);
   1/z is broadcast to [f, h] via a ones-matrix matmul and applied with
   LeakyReLU via one DVE multiply.  Outputs gather in one f16 out-DMA.
"""

import os
import numpy as np
import ml_dtypes

import concourse.bass as bass  # noqa: F401
import concourse.mybir as mybir
import concourse.tile as tile
from concourse import bacc, bass_utils

F32 = mybir.dt.float32
F16 = mybir.dt.float16
BF16 = mybir.dt.bfloat16
FP8 = mybir.dt.float8e4
AF = mybir.ActivationFunctionType
ALU = mybir.AluOpType
DR = mybir.MatmulPerfMode.DoubleRow

H, D, E, F = 8, 256, 1536, 768
B, L = 64, 256
NCORES = 8
GB, GH = 4, 2              # batch groups x head groups
HL = H // GH               # 4 local heads
BPC = B // GB              # 16 b's per core
NBP = BPC // 2             # 8 b-pairs per core
NQ = NBP // 2              # 4 score/softmax quads (4 b's each)
EC, FC, DC = E // 128, F // 128, D // 128   # 12, 6, 2
# Per-bp padded unmasked-column counts (host sorts 64 b's by count; bp_j
# takes ranks [8j, 8j+8)).  Defaults match the fixed harness input.
LPS_DEFAULT = (152, 136, 132, 130, 128, 126, 124, 120)
SK, SB = 64.0, 16.0        # fp8 pre-scales for Key / bank (powers of two)


def _build_program(lps=LPS_DEFAULT):
    assert all(lp % 2 == 0 for lp in lps)
    lhs_ = [lp // 2 for lp in lps]     # l-chunks: two per b
    lpps = [2 * lp for lp in lps]      # (b2, l') columns per (h, dc) group
    lqs = [lhs_[2 * j] for j in range(NQ)]    # quad row counts (max of pair)
    kt_cols = 2 * FC * D               # per-h Key cols ([K8, Kr] streams)
    bkt_cols = [2 * FC * w for w in lpps]     # per-bp bankT cols
    bkt_off = np.cumsum([0] + bkt_cols).tolist()
    # bkn/sbias rows are padded to the quad max so one softmax chain can
    # cover 4 b's; pad rows carry -1e4 bias (exp -> 0) / zero bank rows
    bkn_off = np.cumsum([0] + [2 * lqs[bp // 2] for bp in range(NBP)]).tolist()
    sb_off = np.cumsum([0] + [4 * lqs[bp // 2] for bp in range(NBP)]).tolist()
    tanh_scale = 1.0 / (SK * SB)

    nc = bacc.Bacc("TRN2", target_bir_lowering=False, debug=False,
                   enable_asserts=False, num_devices=NCORES)
    qt = nc.dram_tensor("qt", [HL, 128, EC * D], F16, kind="ExternalInput").ap()
    xt = nc.dram_tensor("xt", [128, EC * BPC], F16, kind="ExternalInput").ap()
    kt = nc.dram_tensor("kt", [HL, 128, kt_cols], FP8, kind="ExternalInput").ap()
    bkt = nc.dram_tensor("bkt", [128, bkt_off[-1]], FP8, kind="ExternalInput").ap()
    bkn = nc.dram_tensor("bkn", [bkn_off[-1], 2 * F], BF16, kind="ExternalInput").ap()
    sbias = nc.dram_tensor("sbias", [1, sb_off[-1]], F32, kind="ExternalInput").ap()
    # out cols: (quad, i4=(bp2, b2), fc, h)
    out = nc.dram_tensor("out", [128, NQ * 4 * FC * HL], F16,
                         kind="ExternalOutput").ap()

    with tile.TileContext(nc) as tc:
        with tc.tile_pool(name="const", bufs=1) as cpool, \
             tc.tile_pool(name="weights", bufs=1) as wpool, \
             tc.tile_pool(name="bktp", bufs=1) as bpool, \
             tc.tile_pool(name="bknp", bufs=1) as npool, \
             tc.tile_pool(name="ksb", bufs=1) as kpool, \
             tc.tile_pool(name="small", bufs=4) as spool, \
             tc.tile_pool(name="psK", bufs=4, space="PSUM") as psK, \
             tc.tile_pool(name="psQ", bufs=1, space="PSUM") as psQ, \
             tc.tile_pool(name="psS", bufs=3, space="PSUM") as psS:

            # ---------------- SBUF tiles ----------------------------------
            xt_sb = cpool.tile([128, EC * BPC], F16)
            kt_sb = [wpool.tile([128, kt_cols], FP8, name=f"kt{h}", tag=f"kt{h}")
                     for h in range(HL)]
            qt_sb = [wpool.tile([128, EC * D], F16, name=f"qt{h}", tag=f"qt{h}")
                     for h in range(HL)]
            bkt_t = [bpool.tile([128, bkt_cols[bp]], FP8,
                                name=f"bkt{bp}", tag=f"bkt{bp}")
                     for bp in range(NBP)]
            bkn_t = [[npool.tile([lqs[bp // 2], 2 * F], BF16,
                                 name=f"bkn{bp}_{b2}", tag=f"bkn{bp}_{b2}")
                      for b2 in range(2)] for bp in range(NBP)]
            sb_sb = cpool.tile([1, sb_off[-1]], F32)
            # f16 output: final values are O(1), so f16 (0.05% rel) halves
            # the tail-critical out-DMA transfers; host upcasts
            o2a = cpool.tile([128, (NQ - 1) * 4 * FC * HL], F16)
            o2b = cpool.tile([128, 4 * FC * HL], F16)
            onesb = cpool.tile([1, BPC], F32)
            ones_mat = cpool.tile([lqs[0], 128], BF16)
            q_sb = cpool.tile([128, 128], F16)

            # ---------------- DMA: priority order -------------------------
            def dma_bkt(bp, s=None):
                o = bkt_off[bp]
                w = bkt_cols[bp]
                if s is None:
                    nc.sync.dma_start(bkt_t[bp][:], bkt[:, o:o + w])
                else:
                    h2 = w // 2
                    nc.sync.dma_start(bkt_t[bp][:, s * h2:(s + 1) * h2],
                                      bkt[:, o + s * h2:o + (s + 1) * h2])

            # kt0 K8-half and bkt0/1 B8-halves first so the warm-phase T1
            # matmuls (k0+k1 interleaved per head) start early
            hk = kt_cols // 2
            nc.sync.dma_start(kt_sb[0][:, 0:hk], kt[0, :, 0:hk])
            dma_bkt(0, 1)
            dma_bkt(1, 1)
            nc.sync.dma_start(kt_sb[0][:, hk:2 * hk], kt[0, :, hk:2 * hk])
            dma_bkt(0, 0)
            dma_bkt(1, 0)
            nc.sync.dma_start(kt_sb[1][:, 0:hk], kt[1, :, 0:hk])
            nc.sync.dma_start(kt_sb[1][:, hk:2 * hk], kt[1, :, hk:2 * hk])
            nc.sync.dma_start(xt_sb[:], xt)
            nc.vector.memset(onesb[:], 1.0)
            nc.vector.memset(ones_mat[:], 1.0)
            nc.sync.dma_start(kt_sb[2][:], kt[2])
            dma_bkt(2)
            nc.sync.dma_start(kt_sb[3][:], kt[3])
            dma_bkt(3)
            nc.sync.dma_start(qt_sb[0][:], qt[0])
            dma_bkt(4)
            nc.sync.dma_start(qt_sb[1][:], qt[1])
            dma_bkt(5)
            nc.sync.dma_start(qt_sb[2][:], qt[2])
            nc.sync.dma_start(qt_sb[3][:], qt[3])
            nc.sync.dma_start(sb_sb[:], sbias)
            dma_bkt(6)
            dma_bkt(7)
            for bp in range(NBP):
                for b2 in range(2):
                    r = bkn_off[bp] + b2 * lqs[bp // 2]
                    nc.sync.dma_start(bkn_t[bp][b2][:],
                                      bkn[r:r + lqs[bp // 2]])

            # ---------------- k = tanh(Key @ bankT), all bps --------------
            k_sb = {}

            def k_phase(bps, warm=False, post_h=None, tail_quad=None):
                def t1_mms(bp, h, ps):
                    lpp = lpps[bp]
                    vb = bkt_t[bp][:].rearrange("p (s ft c) -> p s ft c",
                                                s=2, ft=FC)
                    vk = kt_sb[h][:].rearrange("p (s ft d) -> p s ft d",
                                               s=2, ft=FC)
                    for dc in range(DC):
                        g = ps[dc][:, 0:lpp]
                        for p in range(FC // 2):
                            nc.tensor.matmul(
                                g,
                                vk[:, 0, 2 * p:2 * p + 2,
                                   dc * 128:(dc + 1) * 128],
                                vb[:, 1, 2 * p:2 * p + 2],
                                start=(p == 0), stop=False, perf_mode=DR)

                def cross_evict(bp, h, ps):
                    lpp = lpps[bp]
                    vb = bkt_t[bp][:].rearrange("p (s ft c) -> p s ft c",
                                                s=2, ft=FC)
                    vk = kt_sb[h][:].rearrange("p (s ft d) -> p s ft d",
                                               s=2, ft=FC)
                    for dc in range(DC):
                        g = ps[dc][:, 0:lpp]
                        # cross terms: K8.Br + Kr.B8 per f-tile
                        for ft in range(FC):
                            nc.tensor.matmul(
                                g, vk[:, :, ft, dc * 128:(dc + 1) * 128],
                                vb[:, :, ft],
                                start=False, stop=(ft == FC - 1),
                                perf_mode=DR)
                        # per-dc eviction into a per-dc tile: dc0 drains
                        # while dc1 fills, and (deps being tile-granular)
                        # score matmuls of dc0 can fire before the dc1
                        # eviction lands
                        kt_out = kpool.tile([128, lpp], F16,
                                            name=f"k{bp}_{h}_{dc}",
                                            tag=f"k{bp}_{h}_{dc}")
                        nc.scalar.activation(
                            kt_out[:, 0:lpp],
                            ps[dc][:, 0:lpp],
                            AF.Tanh, scale=tanh_scale)
                        k_sb[(bp, h, dc)] = kt_out

                start_h = 0
                if warm:
                    # four T1 groups lead (they need only the K8/B8
                    # slices; the residual streams for the crosses land
                    # while they run).  The third group borrows a psS
                    # buffer pair (idle until the score phase).
                    b0, b1 = bps[0], bps[1]
                    tA = [psK.tile([128, 512], F32, name="psk", tag="psk")
                          for _ in range(DC)]
                    t1_mms(b0, 0, tA)
                    tB = [psK.tile([128, 512], F32, name="psk", tag="psk")
                          for _ in range(DC)]
                    t1_mms(b1, 0, tB)
                    tC = [psS.tile([128, 512], F32, name="mix", tag="mix")
                          for _ in range(DC)]
                    t1_mms(b0, 1, tC)
                    cross_evict(b0, 0, tA)
                    tD = [psK.tile([128, 512], F32, name="psk", tag="psk")
                          for _ in range(DC)]
                    t1_mms(b1, 1, tD)
                    cross_evict(b1, 0, tB)
                    cross_evict(b0, 1, tC)
                    cross_evict(b1, 1, tD)
                    start_h = 2
                for h in range(start_h, HL):
                    for bp in bps:
                        ps = [psK.tile([128, 512], F32,
                                       name="psk", tag="psk")
                              for _ in range(DC)]
                        t1_mms(bp, h, ps)
                        cross_evict(bp, h, ps)
                    if tail_quad is not None and h >= 1:
                        # the last quad's score matmuls ride one head
                        # behind the joint phase's own evictions
                        qd, qps = tail_quad
                        for bp2 in range(2):
                            score_qh(qd, qps, bp2, h - 1)
                    if post_h and h in post_h:
                        post_h[h]()

            # k0+k1 interleaved per head (2.3us of PE work per kt[h]
            # arrival so the lead-in is never DMA-starved)
            k_phase([0, 1], warm=True)
            k_phase([2])
            k_phase([3])

            # ---------------- q = tanh(Query @ x), transposed -------------
            # qt (3.15 MB) has streamed in behind the k inputs by now
            psq = psQ.tile([128, 512], F32)
            for h in range(HL):
                vq = qt_sb[h][:].rearrange("p (ec d) -> p ec d", ec=EC)
                for dc in range(DC):
                    g = psq[:, (h * DC + dc) * BPC:(h * DC + dc + 1) * BPC]
                    for ec in range(EC):
                        nc.tensor.matmul(
                            g, vq[:, ec, dc * 128:(dc + 1) * 128],
                            xt_sb[:, ec * BPC:(ec + 1) * BPC],
                            start=(ec == 0), stop=(ec == EC - 1))
            nc.scalar.activation(q_sb[:], psq[:, 0:128], AF.Tanh)

            # ---------------- score / softmax / emb per QUAD --------------
            # One chain covers 4 b's (bps 2j, 2j+1), rows padded to the
            # quad max (pad rows get -1e4 bias -> exp 0, zero bank rows).
            # score cols: (i4=(bp2, b2), lc, h) -> 8 groups of HL
            def score_qh(qd, ps, bp2, h):
                """One head's score matmuls for one bp of a quad (+bias
                matmuls at h==0)."""
                lq = lqs[qd]
                bp = 2 * qd + bp2
                lh, lp, lpp = lhs_[bp], lps[bp], lpps[bp]
                for b2 in range(2):
                    i4 = bp2 * 2 + b2
                    for lc in range(2):
                        col = (i4 * 2 + lc) * HL
                        if h == 0:
                            boff = sb_off[bp] + (b2 * 2 + lc) * lq
                            nc.tensor.matmul(ps[0:lq, col:col + HL],
                                             sb_sb[:, boff:boff + lq],
                                             onesb[:, 0:HL],
                                             start=True, stop=False)
                        for dc in range(DC):
                            nc.tensor.matmul(
                                ps[0:lh, col + h:col + h + 1],
                                k_sb[(bp, h, dc)][:, b2 * lp + lc * lh:
                                                  b2 * lp + lc * lh + lh],
                                q_sb[:, (h * DC + dc) * BPC + bp * 2 + b2:
                                     (h * DC + dc) * BPC + bp * 2 + b2 + 1],
                                start=False,
                                stop=(h == HL - 1 and dc == DC - 1))

            def score_exp(qd, ps):
                lq = lqs[qd]
                exp_t = spool.tile([lqs[0], 8 * HL], BF16,
                                   name="exp", tag="exp")
                nc.scalar.activation(exp_t[0:lq, :], ps[0:lq, 0:8 * HL],
                                     AF.Exp)
                return ps, exp_t

            def score_part(qd, ps_tile=None):
                ps = (ps_tile if ps_tile is not None
                      else psS.tile([128, 512], F32, name="mix", tag="mix"))
                for bp2 in range(2):
                    for h in range(HL):
                        score_qh(qd, ps, bp2, h)
                return score_exp(qd, ps)

            def rest_part(qd, ps, exp_t):
                lq = lqs[qd]
                # z[i4, h] (cols 64:80): the ones-MATRIX lhsT emits column
                # sums broadcast across all 128 partitions
                ev = exp_t[0:lq, :].rearrange("p (i4 lc h) -> p i4 lc h",
                                              i4=4, lc=2)
                for lc in range(2):
                    nc.tensor.matmul(ps[:, 64:64 + 4 * HL],
                                     ones_mat[0:lq, :], ev[:, :, lc],
                                     start=(lc == 0), stop=(lc == 1))
                rz = spool.tile([128, 4 * HL], F32, name="rz", tag="rz")
                nc.vector.reciprocal(rz[:], ps[:, 64:64 + 4 * HL])
                # emb[f, (i4, fc, h)] (cols 192:288)
                for bp2 in range(2):
                    bp = 2 * qd + bp2
                    for b2 in range(2):
                        i4 = bp2 * 2 + b2
                        for fc in range(FC):
                            col = 192 + (i4 * FC + fc) * HL
                            for lc in range(2):
                                nc.tensor.matmul(
                                    ps[:, col:col + HL],
                                    bkn_t[bp][b2][:, lc * F + fc * 128:
                                                  lc * F + fc * 128 + 128],
                                    exp_t[0:lq, (i4 * 2 + lc) * HL:
                                          (i4 * 2 + lc + 1) * HL],
                                    start=(lc == 0), stop=(lc == 1))
                o1 = spool.tile([128, 4 * FC * HL], F32, name="o1", tag="o1")
                w = 4 * FC * HL
                o2 = (o2b[:, 0:w] if qd == NQ - 1
                      else o2a[:, qd * w:(qd + 1) * w])
                # LeakyReLU commutes with the positive 1/z: Prelu the raw
                # emb on Act in parallel with the z/recip chain; one DVE
                # multiply finishes
                nc.scalar.activation(o1[:], ps[:, 192:192 + w], AF.Prelu,
                                     alpha=0.4)
                vb = rz[:].rearrange(
                    "p (i4 one h) -> p i4 one h", i4=4,
                    one=1).broadcast_to([128, 4, FC, HL])
                nc.vector.tensor_mul(
                    o2.rearrange("p (i4 fc h) -> p i4 fc h", i4=4, fc=FC),
                    o1[:].rearrange("p (i4 fc h) -> p i4 fc h", i4=4, fc=FC),
                    vb)
                if qd == NQ - 2:
                    # quads 0-2 stream out while quad3 still computes
                    nc.sync.dma_start(out[:, 0:(NQ - 1) * w], o2a[:])
                elif qd == NQ - 1:
                    # only quad3's small transfer sits on the tail
                    nc.sync.dma_start(out[:, (NQ - 1) * w:NQ * w], o2b[:])

            # quads 0/1 score+softmax overlap k4/k5; quad2's inputs finish
            # at k5 so its chain rides inside the joint k6+k7 phase; quad3
            # scores after it (pending/rest pipeline shape)
            k_phase([4])
            k_phase([5])
            pending = None
            for qd in range(NQ - 1):
                cur = (qd, *score_part(qd))
                if pending is not None:
                    rest_part(*pending)
                pending = cur
            k_phase([6, 7], post_h={1: (lambda p=pending: rest_part(*p))},
                    tail_quad=(NQ - 1, psq))
            # tail: last head's scores, exp, softmax/emb/out for quad3
            for bp2 in range(2):
                score_qh(NQ - 1, psq, bp2, HL - 1)
            rest_part(NQ - 1, *score_exp(NQ - 1, psq))

    nc.finalize()
    return nc


def _slot_plan(mask):
    """Sort b's by unmasked count (desc); bp_j takes ranks [8j, 8j+8).
    Returns (perm, lps): perm[slot] = original b, slot = gb*BPC + j*2 + b2."""
    counts = mask.sum(axis=1)
    order = np.argsort(-counts, kind="stable")
    perm = np.empty(B, dtype=np.int64)
    for j in range(NBP):
        grp = order[8 * j:8 * (j + 1)]
        for gb in range(GB):
            perm[gb * BPC + j * 2] = grp[2 * gb]
            perm[gb * BPC + j * 2 + 1] = grp[2 * gb + 1]
    lps = tuple(max(int(2 * ((counts[order[8 * j]] + 1) // 2)), 8)
                for j in range(NBP))
    return perm, lps


def _host_prep(x, bank, mask, Query, Key, perm, lps):
    x = np.asarray(x, dtype=np.float32)
    bank = np.asarray(bank, dtype=np.float32)
    mask = np.asarray(mask)
    Query = np.asarray(Query, dtype=np.float32)
    Key = np.asarray(Key, dtype=np.float32)
    e4 = ml_dtypes.float8_e4m3
    lhs_ = [lp // 2 for lp in lps]
    lqs = [lhs_[2 * j] for j in range(len(lps) // 2)]

    # q path: f16, host-transposed; per head-group slice
    xs = x[perm]
    qt_full = np.ascontiguousarray(Query.transpose(0, 2, 1)).reshape(
        H, EC, 128, D).transpose(0, 2, 1, 3).reshape(H, 128, EC * D)
    qt_full = qt_full.astype(np.float16)

    Ks = Key * SK
    K8 = Ks.astype(e4)
    Kr = (Ks - K8.astype(np.float32)).astype(e4)

    def swz_key(Kt):  # [H, D, F] -> [H, 128(f), FC, D]
        t = np.ascontiguousarray(Kt.transpose(0, 2, 1))
        return t.reshape(H, FC, 128, D).transpose(0, 2, 1, 3)

    kt_full = np.stack([swz_key(K8.astype(np.float32)),
                        swz_key(Kr.astype(np.float32))], axis=2)
    kt_full = kt_full.reshape(H, 128, 2 * FC * D).astype(e4)

    # per-(batch-group, bp) compacted bank streams
    bkt_cols = sum(2 * FC * 2 * lp for lp in lps)
    gb_data = []
    for gb in range(GB):
        bkt_c = np.zeros((128, bkt_cols), dtype=e4)
        bkn_rows = []
        sb_c = []
        col = 0
        for j in range(NBP):
            lp, lh, lq = lps[j], lhs_[j], lqs[j // 2]
            bc = np.zeros((2, lp, F), dtype=np.float32)
            # bias rows padded to the quad max (pad rows -> -1e4)
            bias = np.full((2, 2, lq), -10000.0, dtype=np.float32)
            for b2 in range(2):
                bsrc = perm[gb * BPC + j * 2 + b2]
                idx = np.nonzero(mask[bsrc])[0]
                bc[b2, :len(idx)] = bank[bsrc, idx]
                # column c of this b maps to row c%lh, chunk lc=c//lh;
                # valid rows per lc chunk:
                n = len(idx)
                for lc in range(2):
                    good = max(0, min(lh, n - lc * lh))
                    bias[b2, lc, :good] = 0.0
            # bankT swizzle: [2, lp, F] -> [128(f), s, FC, 2, lp]
            t = np.ascontiguousarray(bc.transpose(0, 2, 1))     # [2, F, lp]
            t = t.reshape(2, FC, 128, lp).transpose(2, 1, 0, 3)  # [128,FC,2,lp]
            ts = t * SB
            t8 = ts.astype(e4)
            tr = (ts - t8.astype(np.float32)).astype(e4)
            blk = np.stack([tr, t8.astype(e4)], axis=1).reshape(
                128, 2 * FC * 2 * lp)
            w = 2 * FC * 2 * lp
            bkt_c[:, col:col + w] = blk
            col += w
            # bkn rows [2*lq, 2F] per bp (b2-major): rows beyond lh zero
            br = np.zeros((2, lq, 2 * F), dtype=np.float32)
            bcq = bc.reshape(2, 2, lh, F)   # [b2, lc, lh, F]
            br[:, :lh, 0:F] = bcq[:, 0]
            br[:, :lh, F:2 * F] = bcq[:, 1]
            bkn_rows.append(br.reshape(2 * lq, 2 * F))
            sb_c.append(bias.reshape(4 * lq))
        xt_gb = np.ascontiguousarray(
            xs[gb * BPC:(gb + 1) * BPC].T.reshape(EC, 128, BPC)
            .transpose(1, 0, 2).reshape(128, EC * BPC)).astype(np.float16)
        gb_data.append({
            "xt": xt_gb,
            "bkt": bkt_c,
            "bkn": np.ascontiguousarray(np.concatenate(bkn_rows, axis=0))
            .astype(ml_dtypes.bfloat16),
            "sbias": np.concatenate(sb_c)[None, :].astype(np.float32),
        })

    in_maps = []
    for c in range(NCORES):
        gb, gh = c // GH, c % GH
        m = dict(gb_data[gb])
        m["qt"] = qt_full[gh * HL:(gh + 1) * HL]
        m["kt"] = kt_full[gh * HL:(gh + 1) * HL]
        in_maps.append(m)
    return in_maps


_NC_CACHE = {}


def kernel(x, bank, mask, Query, Key):
    mask = np.asarray(mask)
    perm, lps = _slot_plan(mask)
    if lps not in _NC_CACHE:
        _NC_CACHE[lps] = _build_program(lps)
    nc = _NC_CACHE[lps]
    in_maps = _host_prep(x, bank, mask, Query, Key, perm, lps)

    trace = os.environ.get("KERNEL_TRACE", "0") == "1"
    res = bass_utils.run_bass_kernel_spmd(nc, in_maps,
                                          core_ids=list(range(NCORES)),
                                          trace=trace)
    full = np.empty((B, H, F), dtype=np.float32)
    for c, r in enumerate(res.results):
        gb, gh = c // GH, c % GH
        a = r["out"].astype(np.float32).reshape(128, NQ, 2, 2, FC, HL)
        # [p, quad, bp2, b2, fc, h] -> slot (j=quad*2+bp2, b2) -> [BPC,HL,F]
        a = a.transpose(1, 2, 3, 5, 4, 0).reshape(BPC, HL, F)
        full[perm[gb * BPC:(gb + 1) * BPC], gh * HL:(gh + 1) * HL] = a
    return np.ascontiguousarray(full)
